# revision 1
# baseline (speedup 1.0000x reference)
"""CPI_DGLLife kernel for 8 Trainium2 NeuronCores (SPMD).

GCN over a 65536-node graph + protein conv1d branch + CPI head.
Sharding: data-parallel over the 512-graph batch (64 graphs / core).
Each core: full h0 table build (replicated), dma_gather edge aggregation
for its dst nodes, fp32r conv stack for its 64 proteins.
"""
import sys
sys.path.insert(0, "/opt/trn_rl_repo")
import contextlib
import numpy as np

import concourse.bass as bass
import concourse.bacc as bacc
import concourse.tile as tile
from concourse import mybir
from concourse.bass_utils import run_bass_kernel_spmd
from concourse.masks import make_identity

dt = mybir.dt
AF = mybir.ActivationFunctionType
ALU = mybir.AluOpType
AX = mybir.AxisListType

P = 128
N, E, B, L = 65536, 262144, 512, 1000
IN_DIM, HID, VOCAB = 74, 128, 25
CHANNELS = [HID, 96, 128, IN_DIM, HID]
NCORES = 8
GPC = B // NCORES              # graphs per core = 64
PPC = GPC                      # proteins per core = 64
# h0 tables: 512-aligned split, local idx = node - base + 1, row 0 = zeros
TBL_BASES = [0, 32256, 64512]
TBL_NNODES = [32256, 32256, 1024]
TBL_ROWS = [n + 1 for n in TBL_NNODES]
TOK_BUDGET = 4096              # max tokens per dma_gather instruction
LCONV = 1002                   # 1000 + 2 guard cols


# ------------------------------------------------------------------ host prep
def _host_prep(inputs):
    graph_ids = np.asarray(inputs["graph_ids"])
    src = np.concatenate([np.asarray(inputs["edge_src"]).astype(np.int64),
                          np.arange(N, dtype=np.int64)])
    dst = np.concatenate([np.asarray(inputs["edge_dst"]).astype(np.int64),
                          np.arange(N, dtype=np.int64)])
    deg_out = np.bincount(src, minlength=N).astype(np.float32)
    deg_in = np.bincount(dst, minlength=N).astype(np.float32)

    core_node_lo = np.searchsorted(graph_ids, np.arange(0, B + 1, GPC))
    ncore_nodes = core_node_lo[1:] - core_node_lo[:-1]
    NT = int(np.ceil(ncore_nodes.max() / P))  # tiles per core (uniform)
    NPAD = NT * P

    # per-core degree-sorted node permutation (padded with -1)
    perm = np.full((NCORES, NPAD), -1, np.int64)
    for c in range(NCORES):
        lo, hi = int(core_node_lo[c]), int(core_node_lo[c + 1])
        order = np.argsort(-deg_in[lo:hi], kind="stable") + lo
        perm[c, :hi - lo] = order

    # deg_in per perm position (pad 1.0), laid out [P, NT] (p, t)
    deg_in_perm = np.ones((NCORES, NPAD), np.float32)
    m = perm >= 0
    deg_in_perm[m] = deg_in[perm[m]]
    deg_in_perm = deg_in_perm.reshape(NCORES, NT, P).transpose(0, 2, 1).copy()

    # S tiles: [NT, P, GPC] graph membership of permuted nodes
    S = np.zeros((NCORES, NT, P, GPC), np.float32)
    for c in range(NCORES):
        pm = perm[c]
        valid = pm >= 0
        g = graph_ids[pm[valid]] - c * GPC
        tt = np.arange(NPAD)[valid] // P
        pp = np.arange(NPAD)[valid] % P
        S[c, tt, pp, g] = 1.0

    # node -> (core, tile-position) in permuted order
    pos_of = np.full(N, -1, np.int64)
    core_of = np.full(N, -1, np.int64)
    for c in range(NCORES):
        pm = perm[c]
        v = pm >= 0
        pos_of[pm[v]] = np.arange(NPAD)[v]
        core_of[pm[v]] = c

    # table id + local row of each node (as gather source)
    tbl_of = np.digitize(np.arange(N), TBL_BASES[1:])
    loc_of = (np.arange(N) - np.asarray(TBL_BASES)[tbl_of] + 1).astype(np.int64)

    # edge placement: core/tile/lane from dst, table/local from src
    ec = core_of[dst]
    et = pos_of[dst] // P
    ep = pos_of[dst] % P
    etbl = tbl_of[src]
    eloc = loc_of[src]

    # slot index within (core, tile, lane, table) group
    key = (((ec * NT + et) * P + ep) * 3 + etbl)
    order = np.argsort(key, kind="stable")
    ks = key[order]
    starts = np.r_[0, np.flatnonzero(np.diff(ks)) + 1]
    grp_len = np.diff(np.r_[starts, E + N])
    slot_sorted = np.arange(E + N) - np.repeat(starts, grp_len)
    slot = np.empty(E + N, np.int64)
    slot[order] = slot_sorted
    # counts per (c, t, p, T) -> kmax per (t, T) across cores/lanes
    cnt = np.zeros(NCORES * NT * P * 3, np.int64)
    uk, uc = np.unique(ks, return_counts=True)
    cnt[uk] = uc
    cnt = cnt.reshape(NCORES, NT, P, 3)
    kmax = cnt.max(axis=2).max(axis=0)  # [NT, 3]

    # gather token schedule per table: tiles packed into instructions
    sched = []  # per table: list of instruction = list of (tile, k)
    for T in range(3):
        instrs, cur, tok = [], [], 0
        for t in range(NT):
            k = int(kmax[t, T])
            if k == 0:
                continue
            if tok + k * P > TOK_BUDGET and cur:
                instrs.append(cur)
                cur, tok = [], 0
            cur.append((t, k))
            tok += k * P
        if cur:
            instrs.append(cur)
        sched.append(instrs)

    # token offset of each tile inside its table stream
    tile_off = np.full((3, NT), 0, np.int64)
    tok_total = [0, 0, 0]
    for T in range(3):
        off = 0
        for ins in sched[T]:
            for (t, k) in ins:
                tile_off[T, t] = off
                off += k * P
        tok_total[T] = max(off, 128)

    idx_flat = [np.zeros((NCORES, tok_total[T]), np.int16) for T in range(3)]
    tok_pos = tile_off[etbl, et] + slot * P + ep
    for T in range(3):
        mT = etbl == T
        idx_flat[T][ec[mT], tok_pos[mT]] = eloc[mT].astype(np.int16)

    def wrap(a):  # token-major -> wrapped [128, tokens//16]
        ncol = a.shape[1] // 16
        w = a.reshape(a.shape[0], ncol, 16).transpose(0, 2, 1)
        return np.ascontiguousarray(np.tile(w, (1, 8, 1)))

    idx_wrapped = [wrap(ix) for ix in idx_flat]

    # per-token deg_out in gather-output layout [128, tokens//128]
    nf = np.asarray(inputs["node_feats"], np.float32)
    tabs = []
    dtok = []
    for T in range(3):
        tb = np.zeros((TBL_ROWS[T], P), np.float32)
        nn = TBL_NNODES[T]
        tb[1:1 + nn, :IN_DIM] = nf[TBL_BASES[T]:TBL_BASES[T] + nn]
        tabs.append(tb)
        d = np.ones((NCORES, tok_total[T]), np.float32)
        mT = etbl == T
        d[ec[mT], tok_pos[mT]] = deg_out[src[mT]]
        dtok.append(np.ascontiguousarray(
            d.reshape(NCORES, tok_total[T] // P, P).transpose(0, 2, 1)))

    # one-hot proteins grouped 4/DMA: [PPC//4, 128, LCONV], p = g*4+s
    seq = np.asarray(inputs["protein_seq"]).reshape(NCORES, PPC, L)
    oh = np.zeros((NCORES, PPC, 32, LCONV), np.float32)
    iot = np.arange(VOCAB)[None, None, :, None]
    oh[:, :, :VOCAB, 1:1 + L] = (seq[:, :, None, :] == iot)
    oh = np.ascontiguousarray(
        oh.reshape(NCORES, PPC // 4, 4 * 32, LCONV))

    shared = {
        "tab0": tabs[0], "tab1": tabs[1], "tab2": tabs[2],
        "W_gc": np.asarray(inputs["W_gc"], np.float32),
        "b_gc": np.asarray(inputs["b_gc"], np.float32).reshape(HID, 1),
        "W_ro_in": np.asarray(inputs["W_ro_in"], np.float32),
        "b_ro_in": np.asarray(inputs["b_ro_in"], np.float32).reshape(HID, 1),
        "W_ro_out": np.asarray(inputs["W_ro_out"], np.float32),
        "b_ro_out": np.asarray(inputs["b_ro_out"], np.float32).reshape(HID, 1),
        "Wc1": np.asarray(inputs["Wc1"], np.float32),
        "bc1": np.asarray(inputs["bc1"], np.float32).reshape(HID, 1),
        "Wc2": np.asarray(inputs["Wc2"], np.float32),
        "bc2": np.asarray(inputs["bc2"], np.float32).reshape(HID, 1),
        "embedT": np.ascontiguousarray(
            np.asarray(inputs["embed"], np.float32).T),       # [HID, 25]
        "Wf1_r": np.ascontiguousarray(
            np.asarray(inputs["Wf1"], np.float32).reshape(2, HID, 2 * HID)),
        "bf1_r": np.ascontiguousarray(
            np.asarray(inputs["bf1"], np.float32).reshape(2, HID, 1)),
        "Wf2_r": np.ascontiguousarray(
            np.asarray(inputs["Wf2"], np.float32).reshape(2, HID, 1)),
        "bf2": np.asarray(inputs["bf2"], np.float32).reshape(1, 1),
    }
    for l in range(4):
        K = np.asarray(inputs["K%d" % (l + 1)], np.float32)  # [o, i, 3]
        shared["K%dT" % (l + 1)] = np.ascontiguousarray(
            K.transpose(1, 2, 0))                            # [i, 3, o]
        shared["cb%d" % (l + 1)] = np.asarray(
            inputs["cb%d" % (l + 1)], np.float32).reshape(-1, 1)

    percore = []
    for c in range(NCORES):
        percore.append({
            "deg_in_perm": np.ascontiguousarray(deg_in_perm[c]),
            "S": np.ascontiguousarray(S[c]),
            "onehot": np.ascontiguousarray(oh[c]),
            "ix0": idx_wrapped[0][c],
            "ix1": idx_wrapped[1][c],
            "ix2": idx_wrapped[2][c],
            "dtok0": dtok[0][c], "dtok1": dtok[1][c], "dtok2": dtok[2][c],
        })
    meta = dict(NT=NT, sched=sched, tok_total=tok_total)
    return shared, percore, meta


# --------------------------------------------------------------- device build
def _build(shared, meta):
    NT = meta["NT"]
    sched = meta["sched"]
    tok_total = meta["tok_total"]

    nc = bacc.Bacc("TRN2", target_bir_lowering=False, debug=False,
                   num_devices=NCORES, num_swdge_queues=4)
    f32, f32r, i16 = dt.float32, dt.float32r, dt.int16

    D = {k: nc.dram_tensor(k, list(v.shape), dt.from_np(v.dtype),
                           kind="ExternalInput")
         for k, v in shared.items()}
    D["deg_in_perm"] = nc.dram_tensor("deg_in_perm", [P, NT], f32,
                                      kind="ExternalInput")
    D["S"] = nc.dram_tensor("S", [NT, P, GPC], f32, kind="ExternalInput")
    D["onehot"] = nc.dram_tensor("onehot", [PPC // 4, P, LCONV], f32,
                                 kind="ExternalInput")
    for T in range(3):
        D["ix%d" % T] = nc.dram_tensor("ix%d" % T, [P, tok_total[T] // 16],
                                       i16, kind="ExternalInput")
    tabs = [D["tab%d" % T] for T in range(3)]
    for T in range(3):
        D["dtok%d" % T] = nc.dram_tensor("dtok%d" % T, [P, tok_total[T] // P],
                                         f32, kind="ExternalInput")
    out_d = nc.dram_tensor("out", [1, GPC], f32, kind="ExternalOutput")

    with tile.TileContext(nc) as tc, contextlib.ExitStack() as ctx:
        wp = ctx.enter_context(tc.tile_pool(name="wp", bufs=1))
        h0p = ctx.enter_context(tc.tile_pool(name="h0p", bufs=3))
        gp = ctx.enter_context(tc.tile_pool(name="gp", bufs=1))
        accp = ctx.enter_context(tc.tile_pool(name="accp", bufs=1))
        cvp = ctx.enter_context(tc.tile_pool(name="cvp", bufs=2))
        gnp = ctx.enter_context(tc.tile_pool(name="gnp", bufs=3))
        pcv = ctx.enter_context(tc.tile_pool(name="pcv", bufs=4, space="PSUM"))
        pgn = ctx.enter_context(tc.tile_pool(name="pgn", bufs=2, space="PSUM"))
        ps1 = ctx.enter_context(tc.tile_pool(name="ps1", bufs=1, space="PSUM"))

        # ---------------- setup: weights to SBUF
        def ld(name, shape, dtype=f32, src=None, tag=None):
            t = wp.tile(shape, dtype, tag=tag or name)
            ap = D[name][:] if src is None else src
            if dtype == f32r:
                ap = ap.bitcast(f32r)
            nc.sync.dma_start(out=t[:], in_=ap)
            return t

        W_gc = ld("W_gc", [IN_DIM, HID], f32r)
        b_gc = ld("b_gc", [HID, 1])
        W_ri = ld("W_ro_in", [HID, HID], f32r); b_ri = ld("b_ro_in", [HID, 1])
        W_ro = ld("W_ro_out", [HID, HID], f32r); b_ro = ld("b_ro_out", [HID, 1])
        Wc1 = ld("Wc1", [HID, HID], f32r); bc1 = ld("bc1", [HID, 1])
        Wc2 = ld("Wc2", [HID, HID], f32r); bc2 = ld("bc2", [HID, 1])
        Wf1 = ld("Wf1_r", [HID, 2, 2 * HID],
                 src=D["Wf1_r"][:].rearrange("k h m -> h k m"))
        bf1 = ld("bf1_r", [HID, 2, 1],
                 src=D["bf1_r"][:].rearrange("k h o -> h k o"))
        Wf2 = ld("Wf2_r", [HID, 2, 1],
                 src=D["Wf2_r"][:].rearrange("k h o -> h k o"))
        bf2 = ld("bf2", [1, 1])
        embT = ld("embedT", [HID, VOCAB], f32r)
        KT = [ld("K%dT" % (l + 1), [CHANNELS[l], 3, CHANNELS[l + 1]], f32r)
              for l in range(4)]
        cb = [ld("cb%d" % (l + 1), [CHANNELS[l + 1], 1]) for l in range(4)]
        Sg = ld("S", [P, NT, GPC], f32r,
                src=D["S"][:].rearrange("t p g -> p t g"))
        ixs = [ld("ix%d" % T, [P, tok_total[T] // 16], i16) for T in range(3)]
        dginp = ld("deg_in_perm", [P, NT])
        dts = [ld("dtok%d" % T, [P, tok_total[T] // P]) for T in range(3)]

        xb = []
        for l in range(3):
            pair = []
            for j in range(2):
                t = wp.tile([CHANNELS[l + 1], LCONV], f32r,
                            tag="xb%d_%d" % (l, j))
                nc.vector.memset(t[:, 0:1].bitcast(dt.float32), 0.0)
                nc.vector.memset(t[:, LCONV - 1:LCONV].bitcast(dt.float32),
                                 0.0)
                pair.append(t)
            xb.append(pair)

        ident = wp.tile([P, P], f32, tag="ident")
        make_identity(nc, ident[:])
        identr = wp.tile([P, P], f32r, tag="identr")
        nc.vector.tensor_copy(identr[:], ident[:])

        # rsqrt factors: w = sqrt(1/deg) per gather token / per dst lane
        for T in range(3):
            nc.vector.reciprocal(dts[T][:], dts[T][:])
            nc.scalar.sqrt(dts[T][:], dts[T][:])
        rdgi = wp.tile([P, NT], f32, tag="rdgi")
        nc.vector.reciprocal(rdgi[:], dginp[:])
        nc.scalar.sqrt(rdgi[:], rdgi[:])

        # M1rep[32s:32s+25, t, :] = embed @ K1_t^T replicated at 4 offsets
        M1rep = wp.tile([P, 3, CHANNELS[1]], f32r, tag="m1rep")
        for t in range(3):
            pm = ps1.tile([VOCAB, CHANNELS[1]], f32, space="PSUM", tag="ps1a")
            nc.tensor.matmul(pm[:], embT[:], KT[0][:, t, :], start=True,
                             stop=True)
            nc.scalar.copy(M1rep[:VOCAB, t, :], pm[:])
        for srow in range(1, 4):
            nc.sync.dma_start(out=M1rep[32 * srow:32 * srow + VOCAB, :, :],
                              in_=M1rep[:VOCAB, :, :])

        # ---------------- interleaved: conv proteins + gather groups
        acc = {}

        def emit_group(grp, after_protein=None):
            ohg = cvp.tile([P, LCONV], f32r, tag="ohg")
            nc.sync.dma_start(out=ohg[:], in_=D["onehot"][grp].bitcast(f32r))
            for srow in range(4):
                p = grp * 4 + srow
                b0 = 32 * srow
                xs = None
                for l in range(4):
                    cin, cout = CHANNELS[l], CHANNELS[l + 1]
                    for cchunk in range(2):
                        c0 = cchunk * 500
                        pps = pcv.tile([cout, 500], f32, space="PSUM",
                                       tag="cps")
                        for tap in range(3):
                            if l == 0:
                                lhsT = M1rep[b0:b0 + VOCAB, tap, :]
                                rhs = ohg[b0:b0 + VOCAB,
                                          c0 + tap:c0 + tap + 500]
                                tpos = (96, 0) if srow == 3 else None
                            else:
                                lhsT = KT[l][:, tap, :]
                                rhs = xs[:cin, c0 + tap:c0 + tap + 500]
                                tpos = None
                            nc.tensor.matmul(pps[:], lhsT, rhs,
                                             start=(tap == 0), stop=(tap == 2),
                                             tile_position=tpos)
                        if l < 3:
                            nc.scalar.activation(
                                xb[l][p % 2][:, 1 + c0:1 + c0 + 500],
                                pps[:], AF.Relu, bias=cb[l][:])
                        else:
                            nc.vector.reduce_max(
                                out=chunkmax[:, cchunk, p:p + 1],
                                in_=pps[:, :500], axis=AX.X)
                    if l < 3:
                        xs = xb[l][p % 2]
                if after_protein is not None:
                    after_protein(p)

        gjobs = []
        for T in range(3):
            off = 0
            for ins in sched[T]:
                gjobs.append((T, off, ins))
                off += sum(k * P for (_, k) in ins)

        def emit_gather(job, qn):
            T, off, ins = job
            ntok = sum(k * P for (_, k) in ins)
            g = gp.tile([P, ntok // P, P], f32, tag="g%d" % (qn % 6))
            nc.gpsimd.dma_gather(
                out_ap=g[:], in_ap=tabs[T][:],
                idxs_ap=ixs[T][:, off // 16:(off + ntok) // 16],
                num_idxs=ntok, num_idxs_reg=ntok, elem_size=P,
                single_packet=False, queue_num=qn % 4)
            blk0 = off // P
            nc.vector.tensor_tensor(
                out=g[:, :, :IN_DIM],
                in0=g[:, :, :IN_DIM],
                in1=dts[T][:, blk0:blk0 + ntok // P, None]
                    .to_broadcast([P, ntok // P, IN_DIM]),
                op=ALU.mult)
            boff = 0
            for (t, k) in ins:
                view = g[:, boff:boff + k, :IN_DIM].rearrange("p k d -> p d k")
                if t not in acc:
                    a = accp.tile([P, IN_DIM], f32, tag="acc%d" % t)
                    acc[t] = a
                    nc.vector.tensor_reduce(out=a[:], in_=view, axis=AX.X,
                                            op=ALU.add)
                else:
                    tmp = gp.tile([P, IN_DIM], f32, tag="rtmp")
                    nc.vector.tensor_reduce(out=tmp[:], in_=view, axis=AX.X,
                                            op=ALU.add)
                    nc.vector.tensor_add(out=acc[t][:], in0=acc[t][:],
                                         in1=tmp[:])
                boff += k

        pmax = wp.tile([P, PPC], f32, tag="pmax")
        chunkmax = wp.tile([P, 2, PPC], f32, tag="chunkmax")
        gq = list(gjobs)
        qst = [0]

        def drain(p):
            while gq and len(gq) > (PPC - 1 - p) * len(gjobs) // PPC:
                emit_gather(gq.pop(0), qst[0])
                qst[0] += 1

        for grp in range(PPC // 4):
            emit_group(grp, after_protein=drain)
        qn = qst[0]
        while gq:
            emit_gather(gq.pop(0), qn)
            qn += 1
        # pmax = relu(max(chunk maxes) + cb4)
        mxt = wp.tile([P, PPC], f32, tag="mxt")
        nc.vector.tensor_reduce(out=mxt[:],
                                in_=chunkmax[:].rearrange("p c q -> p q c"),
                                axis=AX.X, op=ALU.max)
        nc.scalar.activation(pmax[:], mxt[:], AF.Relu, bias=cb[3][:])
        # scale by rsqrt(deg_in)
        for t in range(NT):
            nc.vector.tensor_scalar_mul(acc[t][:], acc[t][:],
                                        rdgi[:, t:t + 1])

        # ---------------- GNN matmul chain (fp32)
        hg_ps = ps1.tile([GPC, HID], f32, space="PSUM", tag="hgps")
        for t in range(NT):
            tp = pgn.tile([IN_DIM, P], f32, space="PSUM", tag="gps")
            nc.tensor.transpose(tp[:], acc[t][:], ident[:])
            aggT = gnp.tile([IN_DIM, P], f32r, tag="aggT")
            nc.scalar.copy(aggT[:], tp[:])
            hps = pgn.tile([HID, P], f32, space="PSUM", tag="gps")
            nc.tensor.matmul(hps[:], W_gc[:], aggT[:], start=True, stop=True)
            h = gnp.tile([HID, P], f32r, tag="h")
            nc.scalar.activation(h[:], hps[:], AF.Relu, bias=b_gc[:])
            x1ps = pgn.tile([HID, P], f32, space="PSUM", tag="gps")
            nc.tensor.matmul(x1ps[:], W_ri[:], h[:], start=True, stop=True)
            x1 = gnp.tile([HID, P], f32r, tag="x1")
            nc.scalar.activation(x1[:], x1ps[:], AF.Identity, bias=b_ri[:])
            x2ps = pgn.tile([HID, P], f32, space="PSUM", tag="gps")
            nc.tensor.matmul(x2ps[:], W_ro[:], x1[:], start=True, stop=True)
            x2 = gnp.tile([HID, P], f32r, tag="x2")
            nc.scalar.activation(x2[:], x2ps[:], AF.Identity, bias=b_ro[:])
            x2t = pgn.tile([P, HID], f32r, space="PSUM", tag="gps")
            nc.tensor.transpose(x2t[:], x2[:], identr[:])
            x2n = gnp.tile([P, HID], f32r, tag="x2n")
            nc.scalar.copy(x2n[:], x2t[:])
            nc.tensor.matmul(hg_ps[:], Sg[:, t, :], x2n[:],
                             start=(t == 0), stop=(t == NT - 1),
                             skip_group_check=True)
        hgT = wp.tile([GPC, HID], f32, tag="hgT")
        nc.scalar.activation(hgT[:], hg_ps[:], AF.Relu)
        hgt_ps = pgn.tile([HID, GPC], f32, space="PSUM", tag="gps")
        nc.tensor.transpose(hgt_ps[:], hgT[:], ident[:GPC, :GPC])
        hg = wp.tile([HID, GPC], f32r, tag="hg")
        nc.scalar.copy(hg[:], hgt_ps[:])
        # compound FC
        c1ps = pgn.tile([HID, GPC], f32, space="PSUM", tag="gps")
        nc.tensor.matmul(c1ps[:], Wc1[:], hg[:], start=True, stop=True)
        cv1 = wp.tile([HID, GPC], f32r, tag="cv1")
        nc.scalar.activation(cv1[:], c1ps[:], AF.Relu, bias=bc1[:])
        c2ps = pgn.tile([HID, GPC], f32, space="PSUM", tag="gps")
        nc.tensor.matmul(c2ps[:], Wc2[:], cv1[:], start=True, stop=True)
        cv2 = wp.tile([HID, GPC], f32, tag="cv2")
        nc.scalar.activation(cv2[:], c2ps[:], AF.Relu, bias=bc2[:])
        # head: z = [cv2; pmax]
        zin = [cv2, pmax]
        z2 = []
        for mc in range(2):
            zps = pgn.tile([HID, GPC], f32, space="PSUM", tag="gps")
            for kc in range(2):
                nc.tensor.matmul(zps[:], Wf1[:, kc, mc * HID:(mc + 1) * HID],
                                 zin[kc][:, :GPC], start=(kc == 0),
                                 stop=(kc == 1))
            zt = wp.tile([HID, GPC], f32, tag="z2_%d" % mc)
            nc.scalar.activation(zt[:], zps[:], AF.Relu, bias=bf1[:, mc, :])
            z2.append(zt)
        ops = ps1.tile([1, GPC], f32, space="PSUM", tag="ps1a")
        for kc in range(2):
            nc.tensor.matmul(ops[:], Wf2[:, kc, :], z2[kc][:],
                             start=(kc == 0), stop=(kc == 1))
        ot = wp.tile([1, GPC], f32, tag="ot")
        nc.scalar.activation(ot[:], ops[:], AF.Sigmoid, bias=bf2[:1, :])
        nc.sync.dma_start(out=out_d[:], in_=ot[:])

    nc.compile()
    return nc


def kernel(**inputs):
    shared, percore, meta = _host_prep(inputs)
    nc = _build(shared, meta)
    in_maps = []
    for c in range(NCORES):
        m = dict(shared)
        m.update(percore[c])
        in_maps.append(m)
    res = run_bass_kernel_spmd(nc, in_maps, list(range(NCORES)))
    out = np.concatenate([res.results[c]["out"].reshape(GPC)
                          for c in range(NCORES)])
    return out.reshape(B, 1).astype(np.float32)


if __name__ == "__main__":
    sys.path.insert(0, "/root/problem")
    import jax
    import reference
    with jax.default_device(jax.devices("cpu")[0]):
        inputs = {k: np.asarray(v) for k, v in reference.setup_inputs().items()}
        exp = np.asarray(reference.reference(**inputs))
    got = kernel(**inputs)
    err = np.abs(got - exp).max()
    rel = err / max(np.abs(exp).max(), 1e-9)
    print("max abs err:", err, " rel:", rel)



# revision 7
# speedup vs baseline: 2.9499x; 2.9499x over previous
"""CPI_DGLLife kernel for 8 Trainium2 NeuronCores (SPMD).

GCN over a 65536-node graph + protein conv1d branch + CPI head.
Sharding: data-parallel over the 512-graph batch (64 graphs / core).

v2 design (vs baseline):
- Single gather stream: bf16 node table packed as 32768 x 512B granules
  (2 node rows each) so int16 indices cover all 65536 nodes. Exact
  per-edge tokens (no 3-table split padding); pad slots carry weight 0.
- Readout commuted past the per-graph segment sum: W_ro_in/W_ro_out are
  applied to the [64, 128] graph sums instead of per node (linear ops
  commute with segment_sum; node-count bias term handled separately).
- bf16 everywhere on the matmul path (fp32 PSUM accumulate), which
  halves gather HBM traffic and runs matmuls at 1 cycle/row.
- Conv restructured: layer-1 folds embed@K1 into a 75-row tap-stacked
  one-hot matmul (1 pass instead of 3); acts split Scalar/Vector.
"""
import sys
sys.path.insert(0, "/opt/trn_rl_repo")
import contextlib
import numpy as np
import ml_dtypes

import concourse.bass as bass
import concourse.bacc as bacc
import concourse.tile as tile
from concourse import mybir
from concourse.bass_utils import run_bass_kernel_spmd
from concourse.masks import make_identity

dt = mybir.dt
AF = mybir.ActivationFunctionType
ALU = mybir.AluOpType
AX = mybir.AxisListType
BF16 = np.dtype(ml_dtypes.bfloat16)

P = 128
N, E, B, L = 65536, 262144, 512, 1000
IN_DIM, HID, VOCAB = 74, 128, 25
CHANNELS = [HID, 96, 128, IN_DIM, HID]
NCORES = 8
GPC = B // NCORES              # graphs per core = 64
PPC = GPC                      # proteins per core = 64
CB = 24                        # gather chunk budget (128-token blocks)
LCONV = 1002                   # 1000 + 2 guard cols


# ------------------------------------------------------------------ host prep
def _host_prep(inputs):
    graph_ids = np.asarray(inputs["graph_ids"])
    src = np.concatenate([np.asarray(inputs["edge_src"]).astype(np.int64),
                          np.arange(N, dtype=np.int64)])
    dst = np.concatenate([np.asarray(inputs["edge_dst"]).astype(np.int64),
                          np.arange(N, dtype=np.int64)])
    deg_out = np.bincount(src, minlength=N).astype(np.float32)
    deg_in = np.bincount(dst, minlength=N).astype(np.float32)

    core_node_lo = np.searchsorted(graph_ids, np.arange(0, B + 1, GPC))
    ncore_nodes = core_node_lo[1:] - core_node_lo[:-1]
    NT = int(np.ceil(ncore_nodes.max() / P))
    NPAD = NT * P

    # per-core degree-sorted node permutation (padded with -1)
    perm = np.full((NCORES, NPAD), -1, np.int64)
    for c in range(NCORES):
        lo, hi = int(core_node_lo[c]), int(core_node_lo[c + 1])
        order = np.argsort(-deg_in[lo:hi], kind="stable") + lo
        perm[c, :hi - lo] = order

    # S tiles: [NT, P, GPC] graph membership of permuted nodes
    S = np.zeros((NCORES, NT, P, GPC), np.float32)
    for c in range(NCORES):
        pm = perm[c]
        valid = pm >= 0
        g = graph_ids[pm[valid]] - c * GPC
        tt = np.arange(NPAD)[valid] // P
        pp = np.arange(NPAD)[valid] % P
        S[c, tt, pp, g] = 1.0

    # node -> (core, tile-position) in permuted order
    pos_of = np.full(N, -1, np.int64)
    core_of = np.full(N, -1, np.int64)
    for c in range(NCORES):
        pm = perm[c]
        v = pm >= 0
        pos_of[pm[v]] = np.arange(NPAD)[v]
        core_of[pm[v]] = c

    # edge -> (core, tile, lane) from dst; slot = rank within (c,t,p)
    ec = core_of[dst]
    et = pos_of[dst] // P
    ep = pos_of[dst] % P
    key = (ec * NT + et) * P + ep
    order = np.argsort(key, kind="stable")
    ks = key[order]
    starts = np.r_[0, np.flatnonzero(np.diff(ks)) + 1]
    grp_len = np.diff(np.r_[starts, len(ks)])
    slot_sorted = np.arange(len(ks)) - np.repeat(starts, grp_len)
    slot = np.empty(len(ks), np.int64)
    slot[order] = slot_sorted
    cnt = np.zeros(NCORES * NT * P, np.int64)
    uk, uc = np.unique(ks, return_counts=True)
    cnt[uk] = uc
    kmax = cnt.reshape(NCORES, NT, P).max(axis=(0, 2))  # [NT] shared schedule
    Bpref = np.r_[0, np.cumsum(kmax)]
    NBLK = int(Bpref[-1])
    NTOK = NBLK * P

    # token (t, slot, lane) -> flat position; idx = src granule, pad -> 0
    tokpos = (Bpref[et] + slot) * P + ep
    idx_flat = np.zeros((NCORES, NTOK), np.int16)
    idx_flat[ec, tokpos] = (src >> 1).astype(np.int16)
    # per-token 2-half weights: deg product at the src half, +inf (w=0) else
    dval = np.full((NCORES, P, NBLK, 2), np.inf, np.float32)
    dval[ec, ep, Bpref[et] + slot, src & 1] = deg_out[src] * deg_in[dst]

    def wrap(a):  # token-major [NCORES, NTOK] -> wrapped [NCORES, 128, NTOK//16]
        ncol = a.shape[1] // 16
        w = a.reshape(NCORES, ncol, 16).transpose(0, 2, 1)
        return np.ascontiguousarray(np.tile(w, (1, 8, 1)))

    idx_wrapped = wrap(idx_flat)

    # gather chunks: pack whole tiles into <= CB blocks per instruction
    chunks = []
    cur, cb, b0 = [], 0, 0
    for t in range(NT):
        k = int(kmax[t])
        if k == 0:
            continue
        if cb + k > CB and cur:
            chunks.append((cur, b0, cb))
            b0 += cb
            cur, cb = [], 0
        cur.append((t, k))
        cb += k
    if cur:
        chunks.append((cur, b0, cb))

    # bf16 node table: 2 rows (2x 128 cols) per 512B granule
    tab = np.zeros((N, P), BF16)
    tab[:, :IN_DIM] = np.asarray(inputs["node_feats"], np.float32)
    tabg = np.ascontiguousarray(tab.reshape(N // 2, 2 * P))

    # tap-stacked protein one-hot: oh3[c, p, 25t+v, j] = [seq[j+t-1] == v]
    seq = np.asarray(inputs["protein_seq"]).reshape(NCORES, PPC, L)
    oh3 = np.zeros((NCORES, PPC, 3 * VOCAB, L), np.float32)
    ci = np.arange(NCORES)[:, None, None]
    pi = np.arange(PPC)[None, :, None]
    for t in range(3):
        j = np.arange(max(0, 1 - t), min(L, L + 1 - t))
        vals = seq[:, :, j + t - 1]
        oh3[ci, pi, VOCAB * t + vals, j[None, None, :]] = 1.0
    oh3 = oh3.reshape(NCORES, PPC // 4, 4, 3 * VOCAB, L)
    oh3 = np.ascontiguousarray(oh3.transpose(0, 1, 3, 2, 4)).reshape(
        NCORES, PPC // 4, 3 * VOCAB, 4 * L).astype(BF16)

    n_g = np.bincount(graph_ids, minlength=B).astype(np.float32)
    n_g = n_g.reshape(NCORES, 1, GPC)

    f32 = np.float32
    shared = {
        "tabg": tabg,
        "W_gc": np.asarray(inputs["W_gc"], f32).astype(BF16),      # [74,128] rhs
        "W_ri": np.asarray(inputs["W_ro_in"], f32).astype(BF16),   # [128,128] lhsT
        "W_ro": np.asarray(inputs["W_ro_out"], f32).astype(BF16),
        "Wc1": np.asarray(inputs["Wc1"], f32).astype(BF16),
        "Wc2": np.asarray(inputs["Wc2"], f32).astype(BF16),
        "embedT": np.ascontiguousarray(
            np.asarray(inputs["embed"], f32).T).astype(BF16),      # [128, 25]
        "Wf1_r": np.ascontiguousarray(
            np.asarray(inputs["Wf1"], f32).reshape(2, HID, 2 * HID)).astype(BF16),
        "bf1_r": np.ascontiguousarray(
            np.asarray(inputs["bf1"], f32).reshape(2, HID, 1)),
        "Wf2_r": np.ascontiguousarray(
            np.asarray(inputs["Wf2"], f32).reshape(2, HID, 1)).astype(BF16),
        "bf2": np.asarray(inputs["bf2"], f32).reshape(1, 1),
        "bc1": np.asarray(inputs["bc1"], f32).reshape(HID, 1),
        "bc2": np.asarray(inputs["bc2"], f32).reshape(HID, 1),
        "bgc_row": np.asarray(inputs["b_gc"], f32).reshape(1, HID).astype(BF16),
        "b1row": np.asarray(inputs["b_ro_in"], f32).reshape(1, HID).astype(BF16),
        "b2row": np.asarray(inputs["b_ro_out"], f32).reshape(1, HID).astype(BF16),
    }
    for l in range(4):
        K = np.asarray(inputs["K%d" % (l + 1)], f32)  # [o, i, 3]
        shared["K%dT" % (l + 1)] = np.ascontiguousarray(
            K.transpose(1, 2, 0)).astype(BF16)        # [i, 3, o]
        shared["cb%d" % (l + 1)] = np.asarray(
            inputs["cb%d" % (l + 1)], f32).reshape(-1, 1)

    percore = []
    for c in range(NCORES):
        percore.append({
            "S": np.ascontiguousarray(S[c]).astype(BF16),
            "oh3": np.ascontiguousarray(oh3[c]),
            "ixs": idx_wrapped[c],
            "dval": np.ascontiguousarray(dval[c]),
            "ngrow": np.ascontiguousarray(n_g[c]),
        })
    gc_bias = bool(np.any(np.asarray(inputs["b_gc"]) != 0))
    ro_bias = bool(np.any(np.asarray(inputs["b_ro_in"]) != 0)
                   or np.any(np.asarray(inputs["b_ro_out"]) != 0))
    meta = dict(NT=NT, NBLK=NBLK, NTOK=NTOK, chunks=chunks,
                gc_bias=gc_bias, ro_bias=ro_bias)
    return shared, percore, meta


# --------------------------------------------------------------- device build
def _build(shared, meta):
    NT = meta["NT"]
    NBLK = meta["NBLK"]
    NTOK = meta["NTOK"]
    chunks = meta["chunks"]
    maxblk = max(cb for (_, _, cb) in chunks)

    nc = bacc.Bacc("TRN2", target_bir_lowering=False, debug=False,
                   num_devices=NCORES, num_swdge_queues=4)
    f32, bf16, i16 = dt.float32, dt.bfloat16, dt.int16

    D = {k: nc.dram_tensor(k, list(v.shape), dt.from_np(v.dtype),
                           kind="ExternalInput")
         for k, v in shared.items()}
    D["S"] = nc.dram_tensor("S", [NT, P, GPC], bf16, kind="ExternalInput")
    D["oh3"] = nc.dram_tensor("oh3", [PPC // 4, 3 * VOCAB, 4 * L], bf16,
                              kind="ExternalInput")
    D["ixs"] = nc.dram_tensor("ixs", [P, NTOK // 16], i16,
                              kind="ExternalInput")
    D["dval"] = nc.dram_tensor("dval", [P, NBLK, 2], f32,
                               kind="ExternalInput")
    D["ngrow"] = nc.dram_tensor("ngrow", [1, GPC], f32,
                                kind="ExternalInput")
    out_d = nc.dram_tensor("out", [1, GPC], f32, kind="ExternalOutput")

    with tile.TileContext(nc) as tc, contextlib.ExitStack() as ctx:
        wp = ctx.enter_context(tc.tile_pool(name="wp", bufs=1))
        cvp = ctx.enter_context(tc.tile_pool(name="cvp", bufs=1))
        gp = ctx.enter_context(tc.tile_pool(name="gp", bufs=1))
        gnp = ctx.enter_context(tc.tile_pool(name="gnp", bufs=2))
        pcv = ctx.enter_context(tc.tile_pool(name="pcv", bufs=5, space="PSUM"))
        pgn = ctx.enter_context(tc.tile_pool(name="pgn", bufs=1, space="PSUM"))
        hgp = ctx.enter_context(tc.tile_pool(name="hgp", bufs=1, space="PSUM"))

        # ---------------- setup: weights to SBUF
        def ld(name, shape, dtype, src=None):
            t = wp.tile(shape, dtype, tag=name)
            nc.sync.dma_start(out=t[:], in_=D[name][:] if src is None else src)
            return t

        W_gc = ld("W_gc", [IN_DIM, HID], bf16)
        W_ri = ld("W_ri", [HID, HID], bf16)
        W_ro = ld("W_ro", [HID, HID], bf16)
        Wc1 = ld("Wc1", [HID, HID], bf16)
        Wc2 = ld("Wc2", [HID, HID], bf16)
        embT = ld("embedT", [HID, VOCAB], bf16)
        Wf1 = ld("Wf1_r", [HID, 2, 2 * HID], bf16,
                 src=D["Wf1_r"][:].rearrange("k h m -> h k m"))
        bf1 = ld("bf1_r", [HID, 2, 1], f32,
                 src=D["bf1_r"][:].rearrange("k h o -> h k o"))
        Wf2 = ld("Wf2_r", [HID, 2, 1], bf16,
                 src=D["Wf2_r"][:].rearrange("k h o -> h k o"))
        bf2 = ld("bf2", [1, 1], f32)
        bc1 = ld("bc1", [HID, 1], f32)
        bc2 = ld("bc2", [HID, 1], f32)
        KT = [ld("K%dT" % (l + 1), [CHANNELS[l], 3, CHANNELS[l + 1]], bf16)
              for l in range(4)]
        cb = [ld("cb%d" % (l + 1), [CHANNELS[l + 1], 1], f32)
              for l in range(4)]
        bgc_row = ld("bgc_row", [1, HID], bf16)
        b1row = ld("b1row", [1, HID], bf16)
        b2row = ld("b2row", [1, HID], bf16)
        ngrow_f = ld("ngrow", [1, GPC], f32)
        Sg = ld("S", [P, NT, GPC], bf16,
                src=D["S"][:].rearrange("t p g -> p t g"))
        ixs = ld("ixs", [P, NTOK // 16], i16)
        dvt = ld("dval", [P, NBLK, 2], f32)

        ngrow = wp.tile([1, GPC], bf16, tag="ngrow_b")
        nc.vector.tensor_copy(ngrow[:], ngrow_f[:])
        ones1 = wp.tile([1, P], bf16, tag="ones1")
        nc.vector.memset(ones1[:], 1.0)

        ident = wp.tile([P, P], f32, tag="ident")
        make_identity(nc, ident[:])
        identb = wp.tile([GPC, GPC], bf16, tag="identb")
        nc.vector.tensor_copy(identb[:], ident[:GPC, :GPC])

        # rsqrt token weights: w = sqrt(1/(deg_out*deg_in)), inf -> 0
        nc.vector.reciprocal(dvt[:], dvt[:])
        nc.scalar.sqrt(dvt[:], dvt[:])
        wz = wp.tile([P, NBLK, 2], bf16, tag="wz")
        nc.vector.tensor_copy(wz[:], dvt[:])

        # L1 stacked weights: rows 25t..25t+24 = embed @ K1_t^T  [75, 96]
        L1w = wp.tile([3 * VOCAB, CHANNELS[1]], bf16, tag="l1w")
        for t in range(3):
            pm = pgn.tile([VOCAB, CHANNELS[1]], f32, space="PSUM", tag="hp")
            nc.tensor.matmul(pm[:], embT[:], KT[0][:, t, :], start=True,
                             stop=True)
            m1t = gnp.tile([VOCAB, CHANNELS[1]], bf16, tag="m1t")
            nc.scalar.copy(m1t[:], pm[:])
            nc.sync.dma_start(out=L1w[VOCAB * t:VOCAB * (t + 1), :],
                              in_=m1t[:])

        # conv x tiles (ping-pong), guard cols zeroed once
        xb = []
        for l in range(3):
            pair = []
            for j in range(2):
                t = cvp.tile([CHANNELS[l + 1], LCONV], bf16,
                             tag="xb%d_%d" % (l, j))
                nc.vector.memset(t[:, 0:1], 0.0)
                nc.vector.memset(t[:, LCONV - 1:LCONV], 0.0)
                pair.append(t)
            xb.append(pair)

        chunkmax = wp.tile([P, 2, PPC], f32, tag="chunkmax")
        acc = {}

        # ---------------- gather machinery
        gtiles = {}

        def emit_gather(j):
            tl, b0, nb = chunks[j]
            g = gp.tile([P, maxblk, 2 * P], bf16, tag="g%d" % (j % 4))
            nc.gpsimd.dma_gather(
                out_ap=g[:, :nb, :], in_ap=D["tabg"][:],
                idxs_ap=ixs[:, b0 * 8:(b0 + nb) * 8],
                num_idxs=nb * P, num_idxs_reg=nb * P, elem_size=2 * P,
                single_packet=False, queue_num=j % 4)
            gtiles[j] = g

        def drain_chunk(j):
            tl, b0, nb = chunks[j]
            g = gtiles[j]
            gv = g[:, :nb, :].rearrange("p k (a d) -> p (k a) d", a=2)
            wv = wz[:, b0:b0 + nb, :].rearrange("p k a -> p (k a)")
            nc.vector.tensor_tensor(
                out=gv[:, :, :IN_DIM], in0=gv[:, :, :IN_DIM],
                in1=wv[:, :, None].to_broadcast([P, 2 * nb, IN_DIM]),
                op=ALU.mult)
            off = 0
            for (t, k) in tl:
                a = wp.tile([P, IN_DIM], f32, tag="acc%d" % t)
                acc[t] = a
                nc.vector.tensor_reduce(
                    out=a[:],
                    in_=gv[:, 2 * off:2 * (off + k), :IN_DIM]
                        .rearrange("p m d -> p d m"),
                    axis=AX.X, op=ALU.add)
                off += k
            if j + 4 < len(chunks):
                emit_gather(j + 4)

        for j in range(min(4, len(chunks))):
            emit_gather(j)
        drain_ptr = [0]

        def drain_due(p):
            while (drain_ptr[0] < len(chunks)
                   and p >= 5 + 3 * drain_ptr[0]):
                drain_chunk(drain_ptr[0])
                drain_ptr[0] += 1

        # ---------------- conv protein loop
        for p in range(PPC):
            grp, sub = p // 4, p % 4
            if sub == 0:
                ohg = cvp.tile([3 * VOCAB, 4 * L], bf16, tag="oh%d" % (grp % 2))
                nc.sync.dma_start(out=ohg[:], in_=D["oh3"][grp])
            base = sub * L
            x1, x2, x3 = xb[0][p % 2], xb[1][p % 2], xb[2][p % 2]
            for ch in range(2):
                c0 = ch * 500
                pp = pcv.tile([CHANNELS[1], 500], f32, space="PSUM", tag="cps")
                nc.tensor.matmul(pp[:], L1w[:], ohg[:, base + c0:base + c0 + 500],
                                 start=True, stop=True)
                nc.scalar.activation(x1[:, 1 + c0:501 + c0], pp[:], AF.Relu,
                                     bias=cb[0][:])
            for ch in range(2):
                c0 = ch * 500
                pp = pcv.tile([CHANNELS[2], 500], f32, space="PSUM", tag="cps")
                for t in range(3):
                    nc.tensor.matmul(pp[:], KT[1][:, t, :],
                                     x1[:, c0 + t:c0 + t + 500],
                                     start=(t == 0), stop=(t == 2))
                nc.vector.tensor_scalar(out=x2[:, 1 + c0:501 + c0], in0=pp[:],
                                        scalar1=cb[1][:], scalar2=0.0,
                                        op0=ALU.add, op1=ALU.max)
            for ch in range(2):
                c0 = ch * 500
                pp = pcv.tile([CHANNELS[3], 500], f32, space="PSUM", tag="cps")
                for t in range(3):
                    nc.tensor.matmul(pp[:], KT[2][:, t, :],
                                     x2[:, c0 + t:c0 + t + 500],
                                     start=(t == 0), stop=(t == 2))
                nc.scalar.activation(x3[:, 1 + c0:501 + c0], pp[:], AF.Relu,
                                     bias=cb[2][:])
            for ch in range(2):
                c0 = ch * 500
                pp = pcv.tile([CHANNELS[4], 500], f32, space="PSUM", tag="cps")
                for t in range(3):
                    nc.tensor.matmul(pp[:], KT[3][:, t, :],
                                     x3[:, c0 + t:c0 + t + 500],
                                     start=(t == 0), stop=(t == 2))
                nc.vector.reduce_max(out=chunkmax[:, ch, p:p + 1],
                                     in_=pp[:, :500], axis=AX.X)
            drain_due(p)

        while drain_ptr[0] < len(chunks):
            drain_chunk(drain_ptr[0])
            drain_ptr[0] += 1

        # pmax = relu(max over positions + cb4)  [128, PPC] bf16
        mxt = wp.tile([P, PPC], f32, tag="mxt")
        nc.vector.tensor_reduce(out=mxt[:],
                                in_=chunkmax[:].rearrange("p c q -> p q c"),
                                axis=AX.X, op=ALU.max)
        pmax = wp.tile([P, PPC], bf16, tag="pmax")
        nc.scalar.activation(pmax[:], mxt[:], AF.Relu, bias=cb[3][:])

        # ---------------- GNN: h = relu(agg @ W_gc + b), graph-sum via S
        hgps = hgp.tile([GPC, HID], f32, space="PSUM", tag="hg")
        for t in range(NT):
            tp = pgn.tile([IN_DIM, P], f32, space="PSUM", tag="tp")
            nc.tensor.transpose(tp[:], acc[t][:], ident[:])
            aT = gnp.tile([IN_DIM, P], bf16, tag="aT")
            nc.scalar.copy(aT[:], tp[:])
            hp = pgn.tile([P, HID], f32, space="PSUM", tag="hp")
            nc.tensor.matmul(hp[:], aT[:], W_gc[:], start=True,
                             stop=not meta["gc_bias"])
            if meta["gc_bias"]:
                nc.tensor.matmul(hp[:], ones1[:], bgc_row[:], start=False,
                                 stop=True)
            h = gnp.tile([P, HID], bf16, tag="h")
            nc.scalar.activation(h[:], hp[:], AF.Relu)
            nc.tensor.matmul(hgps[:], Sg[:, t, :], h[:], start=(t == 0),
                             stop=(t == NT - 1), skip_group_check=True)

        # readout: hg = relu((sum_h @ W_ri + n_g b1) @ W_ro + n_g b2)
        hg_s = gnp.tile([GPC, HID], bf16, tag="hg_s")
        nc.scalar.copy(hg_s[:], hgps[:])
        sT_ps = pgn.tile([HID, GPC], bf16, space="PSUM", tag="tp")
        nc.tensor.transpose(sT_ps[:], hg_s[:], identb[:])
        sT = gnp.tile([HID, GPC], bf16, tag="sT")
        nc.scalar.copy(sT[:], sT_ps[:])
        u_ps = pgn.tile([HID, GPC], f32, space="PSUM", tag="hp")
        nc.tensor.matmul(u_ps[:], W_ri[:], sT[:], start=True,
                         stop=not meta["ro_bias"])
        if meta["ro_bias"]:
            nc.tensor.matmul(u_ps[:], b1row[:], ngrow[:], start=False,
                             stop=True)
        u = gnp.tile([HID, GPC], bf16, tag="u")
        nc.scalar.copy(u[:], u_ps[:])
        v_ps = pgn.tile([HID, GPC], f32, space="PSUM", tag="hp")
        nc.tensor.matmul(v_ps[:], W_ro[:], u[:], start=True,
                         stop=not meta["ro_bias"])
        if meta["ro_bias"]:
            nc.tensor.matmul(v_ps[:], b2row[:], ngrow[:], start=False,
                             stop=True)
        hg = gnp.tile([HID, GPC], bf16, tag="hgv")
        nc.scalar.activation(hg[:], v_ps[:], AF.Relu)
        # compound FC
        c1ps = pgn.tile([HID, GPC], f32, space="PSUM", tag="hp")
        nc.tensor.matmul(c1ps[:], Wc1[:], hg[:], start=True, stop=True)
        cv1 = gnp.tile([HID, GPC], bf16, tag="cv1")
        nc.scalar.activation(cv1[:], c1ps[:], AF.Relu, bias=bc1[:])
        c2ps = pgn.tile([HID, GPC], f32, space="PSUM", tag="hp")
        nc.tensor.matmul(c2ps[:], Wc2[:], cv1[:], start=True, stop=True)
        cv2 = gnp.tile([HID, GPC], bf16, tag="cv2")
        nc.scalar.activation(cv2[:], c2ps[:], AF.Relu, bias=bc2[:])
        # CPI head
        zin = [cv2, pmax]
        z2 = []
        for mc in range(2):
            zps = pgn.tile([HID, GPC], f32, space="PSUM", tag="hp")
            for kc in range(2):
                nc.tensor.matmul(zps[:], Wf1[:, kc, mc * HID:(mc + 1) * HID],
                                 zin[kc][:, :GPC], start=(kc == 0),
                                 stop=(kc == 1))
            zt = gnp.tile([HID, GPC], bf16, tag="z2_%d" % mc)
            nc.scalar.activation(zt[:], zps[:], AF.Relu, bias=bf1[:, mc, :])
            z2.append(zt)
        ops = pgn.tile([1, GPC], f32, space="PSUM", tag="hp")
        for kc in range(2):
            nc.tensor.matmul(ops[:], Wf2[:, kc, :], z2[kc][:],
                             start=(kc == 0), stop=(kc == 1))
        ot = wp.tile([1, GPC], f32, tag="ot")
        nc.scalar.activation(ot[:], ops[:], AF.Sigmoid, bias=bf2[:1, :])
        nc.sync.dma_start(out=out_d[:], in_=ot[:])

    nc.compile()
    return nc


def kernel(**inputs):
    shared, percore, meta = _host_prep(inputs)
    nc = _build(shared, meta)
    in_maps = []
    for c in range(NCORES):
        m = dict(shared)
        m.update(percore[c])
        in_maps.append(m)
    res = run_bass_kernel_spmd(nc, in_maps, list(range(NCORES)))
    out = np.concatenate([res.results[c]["out"].reshape(GPC)
                          for c in range(NCORES)])
    return out.reshape(B, 1).astype(np.float32)


if __name__ == "__main__":
    sys.path.insert(0, "/root/problem")
    import jax
    import reference
    with jax.default_device(jax.devices("cpu")[0]):
        inputs = {k: np.asarray(v) for k, v in reference.setup_inputs().items()}
        exp = np.asarray(reference.reference(**inputs))
    got = kernel(**inputs)
    err = np.abs(got - exp).max()
    rel = err / max(np.abs(exp).max(), 1e-9)
    print("max abs err:", err, " rel:", rel)


# revision 18
# speedup vs baseline: 3.1059x; 1.0529x over previous
"""CPI_DGLLife kernel for 8 Trainium2 NeuronCores (SPMD).

GCN over a 65536-node graph + protein conv1d branch + CPI head.
Sharding: data-parallel over the 512-graph batch (64 graphs / core).

v2 design (vs baseline):
- Single gather stream: bf16 node table packed as 32768 x 512B granules
  (2 node rows each) so int16 indices cover all 65536 nodes. Exact
  per-edge tokens (no 3-table split padding); pad slots carry weight 0.
- Readout commuted past the per-graph segment sum: W_ro_in/W_ro_out are
  applied to the [64, 128] graph sums instead of per node (linear ops
  commute with segment_sum; node-count bias term handled separately).
- bf16 everywhere on the matmul path (fp32 PSUM accumulate), which
  halves gather HBM traffic and runs matmuls at 1 cycle/row.
- Conv restructured: layer-1 folds embed@K1 into a 75-row tap-stacked
  one-hot matmul (1 pass instead of 3); acts split Scalar/Vector.
"""
import sys
sys.path.insert(0, "/opt/trn_rl_repo")
import contextlib
import numpy as np
import ml_dtypes

import concourse.bass as bass
import concourse.bacc as bacc
import concourse.tile as tile
from concourse import mybir
from concourse.bass_utils import run_bass_kernel_spmd
from concourse.masks import make_identity

dt = mybir.dt
AF = mybir.ActivationFunctionType
ALU = mybir.AluOpType
AX = mybir.AxisListType
BF16 = np.dtype(ml_dtypes.bfloat16)

P = 128
N, E, B, L = 65536, 262144, 512, 1000
IN_DIM, HID, VOCAB = 74, 128, 25
CHANNELS = [HID, 96, 128, IN_DIM, HID]
NCORES = 8
GPC = B // NCORES              # graphs per core = 64
PPC = GPC                      # proteins per core = 64
CB = 24                        # gather chunk budget (128-token blocks)
LCONV = 1002                   # 1000 + 2 guard cols


# ------------------------------------------------------------------ host prep
def _host_prep(inputs):
    graph_ids = np.asarray(inputs["graph_ids"])
    src = np.concatenate([np.asarray(inputs["edge_src"]).astype(np.int64),
                          np.arange(N, dtype=np.int64)])
    dst = np.concatenate([np.asarray(inputs["edge_dst"]).astype(np.int64),
                          np.arange(N, dtype=np.int64)])
    deg_out = np.bincount(src, minlength=N).astype(np.float32)
    deg_in = np.bincount(dst, minlength=N).astype(np.float32)

    core_node_lo = np.searchsorted(graph_ids, np.arange(0, B + 1, GPC))
    ncore_nodes = core_node_lo[1:] - core_node_lo[:-1]
    NT = int(np.ceil(ncore_nodes.max() / P))
    NPAD = NT * P

    # per-core degree-sorted node permutation (padded with -1)
    perm = np.full((NCORES, NPAD), -1, np.int64)
    for c in range(NCORES):
        lo, hi = int(core_node_lo[c]), int(core_node_lo[c + 1])
        order = np.argsort(-deg_in[lo:hi], kind="stable") + lo
        perm[c, :hi - lo] = order

    # S tiles: [NT, P, GPC] graph membership of permuted nodes
    S = np.zeros((NCORES, NT, P, GPC), np.float32)
    for c in range(NCORES):
        pm = perm[c]
        valid = pm >= 0
        g = graph_ids[pm[valid]] - c * GPC
        tt = np.arange(NPAD)[valid] // P
        pp = np.arange(NPAD)[valid] % P
        S[c, tt, pp, g] = 1.0

    # node -> (core, tile-position) in permuted order
    pos_of = np.full(N, -1, np.int64)
    core_of = np.full(N, -1, np.int64)
    for c in range(NCORES):
        pm = perm[c]
        v = pm >= 0
        pos_of[pm[v]] = np.arange(NPAD)[v]
        core_of[pm[v]] = c

    # edge -> (core, tile, lane) from dst; slot = rank within (c,t,p)
    ec = core_of[dst]
    et = pos_of[dst] // P
    ep = pos_of[dst] % P
    key = (ec * NT + et) * P + ep
    order = np.argsort(key, kind="stable")
    ks = key[order]
    starts = np.r_[0, np.flatnonzero(np.diff(ks)) + 1]
    grp_len = np.diff(np.r_[starts, len(ks)])
    slot_sorted = np.arange(len(ks)) - np.repeat(starts, grp_len)
    slot = np.empty(len(ks), np.int64)
    slot[order] = slot_sorted
    cnt = np.zeros(NCORES * NT * P, np.int64)
    uk, uc = np.unique(ks, return_counts=True)
    cnt[uk] = uc
    kmax = cnt.reshape(NCORES, NT, P).max(axis=(0, 2))  # [NT] shared schedule
    Bpref = np.r_[0, np.cumsum(kmax)]
    NBLK = int(Bpref[-1])
    NTOK = NBLK * P

    # token (t, slot, lane) -> flat position; idx = src granule, pad -> 0
    tokpos = (Bpref[et] + slot) * P + ep
    idx_flat = np.zeros((NCORES, NTOK), np.int16)
    idx_flat[ec, tokpos] = (src >> 1).astype(np.int16)
    # per-token 2-half weights: deg product at the src half, +inf (w=0) else
    dval = np.full((NCORES, P, NBLK, 2), 1e30, np.float32)
    dval[ec, ep, Bpref[et] + slot, src & 1] = deg_out[src] * deg_in[dst]

    def wrap(a):  # token-major [NCORES, NTOK] -> wrapped [NCORES, 128, NTOK//16]
        ncol = a.shape[1] // 16
        w = a.reshape(NCORES, ncol, 16).transpose(0, 2, 1)
        return np.ascontiguousarray(np.tile(w, (1, 8, 1)))

    idx_wrapped = wrap(idx_flat)

    # gather chunks: pack whole tiles into <= CB blocks per instruction
    chunks = []
    cur, cb, b0 = [], 0, 0
    for t in range(NT):
        k = int(kmax[t])
        if k == 0:
            continue
        if cb + k > CB and cur:
            chunks.append((cur, b0, cb))
            b0 += cb
            cur, cb = [], 0
        cur.append((t, k))
        cb += k
    if cur:
        chunks.append((cur, b0, cb))

    # bf16 node table: 2 rows (2x 128 cols) per 512B granule
    tab = np.zeros((N, P), BF16)
    tab[:, :IN_DIM] = np.asarray(inputs["node_feats"], np.float32)
    tabg = np.ascontiguousarray(tab.reshape(N // 2, 2 * P))

    # tap-stacked protein one-hot: oh3[c, p, 25t+v, j] = [seq[j+t-1] == v]
    seq = np.asarray(inputs["protein_seq"]).reshape(NCORES, PPC, L)
    oh3 = np.zeros((NCORES, PPC, 3 * VOCAB, L), np.float32)
    ci = np.arange(NCORES)[:, None, None]
    pi = np.arange(PPC)[None, :, None]
    for t in range(3):
        j = np.arange(max(0, 1 - t), min(L, L + 1 - t))
        vals = seq[:, :, j + t - 1]
        oh3[ci, pi, VOCAB * t + vals, j[None, None, :]] = 1.0
    oh3 = oh3.reshape(NCORES, PPC // 4, 4, 3 * VOCAB, L)
    oh3 = np.ascontiguousarray(oh3.transpose(0, 1, 3, 2, 4)).reshape(
        NCORES, PPC // 4, 3 * VOCAB, 4 * L).astype(BF16)

    n_g = np.bincount(graph_ids, minlength=B).astype(np.float32)
    n_g = n_g.reshape(NCORES, 1, GPC)

    # pre-arranged for contiguous DMA
    S_r = np.ascontiguousarray(S.transpose(0, 2, 1, 3))  # [c, P, NT, GPC]

    f32 = np.float32
    shared = {
        "tabg": tabg,
        "W_gc": np.asarray(inputs["W_gc"], f32).astype(BF16),      # [74,128] rhs
        "W_ri": np.asarray(inputs["W_ro_in"], f32).astype(BF16),   # [128,128] lhsT
        "W_ro": np.asarray(inputs["W_ro_out"], f32).astype(BF16),
        "Wc1": np.asarray(inputs["Wc1"], f32).astype(BF16),
        "Wc2": np.asarray(inputs["Wc2"], f32).astype(BF16),
        "embedT": np.ascontiguousarray(
            np.asarray(inputs["embed"], f32).T).astype(BF16),      # [128, 25]
        "Wf1_r": np.ascontiguousarray(
            np.asarray(inputs["Wf1"], f32).reshape(2, HID, 2 * HID)
            .transpose(1, 0, 2)).astype(BF16),                     # [HID,2,2H]
        "bf1_r": np.ascontiguousarray(
            np.asarray(inputs["bf1"], f32).reshape(2, HID, 1)
            .transpose(1, 0, 2)),
        "Wf2_r": np.ascontiguousarray(
            np.asarray(inputs["Wf2"], f32).reshape(2, HID, 1)
            .transpose(1, 0, 2)).astype(BF16),
        "bf2": np.asarray(inputs["bf2"], f32).reshape(1, 1),
        "bc1": np.asarray(inputs["bc1"], f32).reshape(HID, 1),
        "bc2": np.asarray(inputs["bc2"], f32).reshape(HID, 1),
        "bgc_row": np.asarray(inputs["b_gc"], f32).reshape(1, HID).astype(BF16),
        "b1row": np.asarray(inputs["b_ro_in"], f32).reshape(1, HID).astype(BF16),
        "b2row": np.asarray(inputs["b_ro_out"], f32).reshape(1, HID).astype(BF16),
    }
    for l in range(4):
        K = np.asarray(inputs["K%d" % (l + 1)], f32)  # [o, i, 3]
        shared["K%dT" % (l + 1)] = np.ascontiguousarray(
            K.transpose(1, 2, 0)).astype(BF16)        # [i, 3, o]
        shared["cb%d" % (l + 1)] = np.asarray(
            inputs["cb%d" % (l + 1)], f32).reshape(-1, 1)

    percore = []
    for c in range(NCORES):
        percore.append({
            "S": np.ascontiguousarray(S_r[c]).astype(BF16),
            "oh3": np.ascontiguousarray(oh3[c]),
            "ixs": idx_wrapped[c],
            "dval": np.ascontiguousarray(dval[c]),
            "ngrow": np.ascontiguousarray(n_g[c]),
        })
    gc_bias = bool(np.any(np.asarray(inputs["b_gc"]) != 0))
    ro_bias = bool(np.any(np.asarray(inputs["b_ro_in"]) != 0)
                   or np.any(np.asarray(inputs["b_ro_out"]) != 0))
    meta = dict(NT=NT, NBLK=NBLK, NTOK=NTOK, chunks=chunks,
                gc_bias=gc_bias, ro_bias=ro_bias)
    return shared, percore, meta


# --------------------------------------------------------------- device build
def _build(shared, meta):
    NT = meta["NT"]
    NBLK = meta["NBLK"]
    NTOK = meta["NTOK"]
    chunks = meta["chunks"]
    maxblk = max(cb for (_, _, cb) in chunks)

    nc = bacc.Bacc("TRN2", target_bir_lowering=False, debug=False,
                   num_devices=NCORES, num_swdge_queues=4)
    f32, bf16, i16 = dt.float32, dt.bfloat16, dt.int16

    D = {k: nc.dram_tensor(k, list(v.shape), dt.from_np(v.dtype),
                           kind="ExternalInput")
         for k, v in shared.items()}
    D["S"] = nc.dram_tensor("S", [P, NT, GPC], bf16, kind="ExternalInput")
    D["oh3"] = nc.dram_tensor("oh3", [PPC // 4, 3 * VOCAB, 4 * L], bf16,
                              kind="ExternalInput")
    D["ixs"] = nc.dram_tensor("ixs", [P, NTOK // 16], i16,
                              kind="ExternalInput")
    D["dval"] = nc.dram_tensor("dval", [P, NBLK, 2], f32,
                               kind="ExternalInput")
    D["ngrow"] = nc.dram_tensor("ngrow", [1, GPC], f32,
                                kind="ExternalInput")
    out_d = nc.dram_tensor("out", [1, GPC], f32, kind="ExternalOutput")

    with tile.TileContext(nc) as tc, contextlib.ExitStack() as ctx:
        wp = ctx.enter_context(tc.tile_pool(name="wp", bufs=1))
        cvp = ctx.enter_context(tc.tile_pool(name="cvp", bufs=1))
        gp = ctx.enter_context(tc.tile_pool(name="gp", bufs=1))
        gnp = ctx.enter_context(tc.tile_pool(name="gnp", bufs=2))
        pcv = ctx.enter_context(tc.tile_pool(name="pcv", bufs=5, space="PSUM"))
        pgn = ctx.enter_context(tc.tile_pool(name="pgn", bufs=1, space="PSUM"))
        hgp = ctx.enter_context(tc.tile_pool(name="hgp", bufs=1, space="PSUM"))

        # ---------------- setup: weights to SBUF
        def ld(name, shape, dtype, src=None):
            t = wp.tile(shape, dtype, tag=name)
            nc.sync.dma_start(out=t[:], in_=D[name][:] if src is None else src)
            return t

        # gather-critical loads first so DGE can start immediately
        ixs = ld("ixs", [P, NTOK // 16], i16)
        dvt = ld("dval", [P, NBLK, 2], f32)
        embT = ld("embedT", [HID, VOCAB], bf16)
        KT = [ld("K%dT" % (l + 1), [CHANNELS[l], 3, CHANNELS[l + 1]], bf16)
              for l in range(4)]
        cb = [ld("cb%d" % (l + 1), [CHANNELS[l + 1], 1], f32)
              for l in range(4)]

        # conv x tiles (ping-pong), guard cols zeroed once (vector queue head)
        xb = []
        for l in range(3):
            pair = []
            for j in range(2):
                t = cvp.tile([CHANNELS[l + 1], LCONV], bf16,
                             tag="xb%d_%d" % (l, j))
                nc.vector.memset(t[:, 0:1], 0.0)
                nc.vector.memset(t[:, LCONV - 1:LCONV], 0.0)
                pair.append(t)
            xb.append(pair)

        # token weights: w = rsqrt(deg_out*deg_in); pad 1e30 -> ~0
        wz = wp.tile([P, NBLK, 2], bf16, tag="wz")
        nc.vector.reciprocal(dvt[:], dvt[:])
        nc.scalar.activation(wz[:], dvt[:], AF.Sqrt)

        # L1 stacked weights: rows 25t..25t+24 = embed @ K1_t^T  [75, 96]
        L1w = wp.tile([3 * VOCAB, CHANNELS[1]], bf16, tag="l1w")
        for t in range(3):
            pm = pgn.tile([VOCAB, CHANNELS[1]], f32, space="PSUM", tag="hp")
            nc.tensor.matmul(pm[:], embT[:], KT[0][:, t, :], start=True,
                             stop=True)
            m1t = gnp.tile([VOCAB, CHANNELS[1]], bf16, tag="m1t")
            nc.scalar.copy(m1t[:], pm[:])
            nc.sync.dma_start(out=L1w[VOCAB * t:VOCAB * (t + 1), :],
                              in_=m1t[:])

        chunkmax = wp.tile([P, 2, PPC], f32, tag="chunkmax")
        acc = {}

        # ---------------- gather machinery
        gtiles = {}

        def emit_gather(j):
            tl, b0, nb = chunks[j]
            g = gp.tile([P, maxblk, 2 * P], bf16, tag="g%d" % (j % 4))
            nc.gpsimd.dma_gather(
                out_ap=g[:, :nb, :], in_ap=D["tabg"][:],
                idxs_ap=ixs[:, b0 * 8:(b0 + nb) * 8],
                num_idxs=nb * P, num_idxs_reg=nb * P, elem_size=2 * P,
                single_packet=False, queue_num=j % 4)
            gtiles[j] = g

        def drain_chunk(j):
            tl, b0, nb = chunks[j]
            g = gtiles[j]
            gv = g[:, :nb, :].rearrange("p k (a d) -> p (k a) d", a=2)
            wv = wz[:, b0:b0 + nb, :].rearrange("p k a -> p (k a)")
            nc.vector.tensor_tensor(
                out=gv[:], in0=gv[:],
                in1=wv[:, :, None].to_broadcast([P, 2 * nb, P]),
                op=ALU.mult)
            off = 0
            for (t, k) in tl:
                a = wp.tile([P, IN_DIM], f32, tag="acc%d" % t)
                acc[t] = a
                nc.vector.tensor_reduce(
                    out=a[:],
                    in_=gv[:, 2 * off:2 * (off + k), :IN_DIM]
                        .rearrange("p m d -> p d m"),
                    axis=AX.X, op=ALU.add)
                off += k
            if j + 4 < len(chunks):
                emit_gather(j + 4)

        for j in range(min(4, len(chunks))):
            emit_gather(j)
        drain_ptr = [0]

        def drain_due(p):
            while (drain_ptr[0] < len(chunks)
                   and p >= 8 + 3 * drain_ptr[0]):
                drain_chunk(drain_ptr[0])
                drain_ptr[0] += 1

        # ---------------- conv protein loop
        for p in range(PPC):
            grp, sub = p // 4, p % 4
            if sub == 0:
                ohg = cvp.tile([3 * VOCAB, 4 * L], bf16, tag="oh%d" % (grp % 2))
                nc.sync.dma_start(out=ohg[:], in_=D["oh3"][grp])
            base = sub * L
            x1, x2, x3 = xb[0][p % 2], xb[1][p % 2], xb[2][p % 2]
            for ch in range(2):
                c0 = ch * 500
                pp = pcv.tile([CHANNELS[1], 500], f32, space="PSUM", tag="cps")
                nc.tensor.matmul(pp[:], L1w[:], ohg[:, base + c0:base + c0 + 500],
                                 start=True, stop=True)
                nc.scalar.activation(x1[:, 1 + c0:501 + c0], pp[:], AF.Relu,
                                     bias=cb[0][:])
            for ch in range(2):
                c0 = ch * 500
                pp = pcv.tile([CHANNELS[2], 500], f32, space="PSUM", tag="cps")
                for t in range(3):
                    nc.tensor.matmul(pp[:], KT[1][:, t, :],
                                     x1[:, c0 + t:c0 + t + 500],
                                     start=(t == 0), stop=(t == 2))
                if p % 2 == 0:
                    nc.vector.tensor_scalar(out=x2[:, 1 + c0:501 + c0],
                                            in0=pp[:], scalar1=cb[1][:],
                                            scalar2=0.0, op0=ALU.add,
                                            op1=ALU.max)
                else:
                    nc.scalar.activation(x2[:, 1 + c0:501 + c0], pp[:],
                                         AF.Relu, bias=cb[1][:])
            for ch in range(2):
                c0 = ch * 500
                pp = pcv.tile([CHANNELS[3], 500], f32, space="PSUM", tag="cps")
                for t in range(3):
                    nc.tensor.matmul(pp[:], KT[2][:, t, :],
                                     x2[:, c0 + t:c0 + t + 500],
                                     start=(t == 0), stop=(t == 2))
                nc.scalar.activation(x3[:, 1 + c0:501 + c0], pp[:], AF.Relu,
                                     bias=cb[2][:])
            for ch in range(2):
                c0 = ch * 500
                pp = pcv.tile([CHANNELS[4], 500], f32, space="PSUM", tag="cps")
                for t in range(3):
                    nc.tensor.matmul(pp[:], KT[3][:, t, :],
                                     x3[:, c0 + t:c0 + t + 500],
                                     start=(t == 0), stop=(t == 2))
                nc.vector.reduce_max(out=chunkmax[:, ch, p:p + 1],
                                     in_=pp[:, :500], axis=AX.X)
            drain_due(p)

        while drain_ptr[0] < len(chunks):
            drain_chunk(drain_ptr[0])
            drain_ptr[0] += 1

        # late loads: needed only by the GNN/readout/head phase
        W_gc = ld("W_gc", [IN_DIM, HID], bf16)
        W_ri = ld("W_ri", [HID, HID], bf16)
        W_ro = ld("W_ro", [HID, HID], bf16)
        Wc1 = ld("Wc1", [HID, HID], bf16)
        Wc2 = ld("Wc2", [HID, HID], bf16)
        Wf1 = ld("Wf1_r", [HID, 2, 2 * HID], bf16)
        bf1 = ld("bf1_r", [HID, 2, 1], f32)
        Wf2 = ld("Wf2_r", [HID, 2, 1], bf16)
        bf2 = ld("bf2", [1, 1], f32)
        bc1 = ld("bc1", [HID, 1], f32)
        bc2 = ld("bc2", [HID, 1], f32)
        bgc_row = ld("bgc_row", [1, HID], bf16)
        b1row = ld("b1row", [1, HID], bf16)
        b2row = ld("b2row", [1, HID], bf16)
        ngrow_f = ld("ngrow", [1, GPC], f32)
        Sg = ld("S", [P, NT, GPC], bf16)
        ngrow = wp.tile([1, GPC], bf16, tag="ngrow_b")
        nc.scalar.copy(ngrow[:], ngrow_f[:])
        ones1 = wp.tile([1, P], bf16, tag="ones1")
        nc.vector.memset(ones1[:], 1.0)
        ident = wp.tile([P, P], f32, tag="ident")
        make_identity(nc, ident[:])
        identb = wp.tile([GPC, GPC], bf16, tag="identb")
        nc.scalar.copy(identb[:], ident[:GPC, :GPC])

        # pmax = relu(max over positions + cb4)  [128, PPC] bf16
        mxt = wp.tile([P, PPC], f32, tag="mxt")
        nc.vector.tensor_reduce(out=mxt[:],
                                in_=chunkmax[:].rearrange("p c q -> p q c"),
                                axis=AX.X, op=ALU.max)
        pmax = wp.tile([P, PPC], bf16, tag="pmax")
        nc.scalar.activation(pmax[:], mxt[:], AF.Relu, bias=cb[3][:])

        # ---------------- GNN: h = relu(agg @ W_gc + b), graph-sum via S
        hgps = hgp.tile([GPC, HID], f32, space="PSUM", tag="hg")
        for t in range(NT):
            tp = pgn.tile([IN_DIM, P], f32, space="PSUM", tag="tp")
            nc.tensor.transpose(tp[:], acc[t][:], ident[:])
            aT = gnp.tile([IN_DIM, P], bf16, tag="aT")
            nc.scalar.copy(aT[:], tp[:])
            hp = pgn.tile([P, HID], f32, space="PSUM", tag="hp")
            nc.tensor.matmul(hp[:], aT[:], W_gc[:], start=True,
                             stop=not meta["gc_bias"])
            if meta["gc_bias"]:
                nc.tensor.matmul(hp[:], ones1[:], bgc_row[:], start=False,
                                 stop=True)
            h = gnp.tile([P, HID], bf16, tag="h")
            nc.scalar.activation(h[:], hp[:], AF.Relu)
            nc.tensor.matmul(hgps[:], Sg[:, t, :], h[:], start=(t == 0),
                             stop=(t == NT - 1), skip_group_check=True)

        # readout: hg = relu((sum_h @ W_ri + n_g b1) @ W_ro + n_g b2)
        hg_s = gnp.tile([GPC, HID], bf16, tag="hg_s")
        nc.scalar.copy(hg_s[:], hgps[:])
        sT_ps = pgn.tile([HID, GPC], bf16, space="PSUM", tag="tp")
        nc.tensor.transpose(sT_ps[:], hg_s[:], identb[:])
        sT = gnp.tile([HID, GPC], bf16, tag="sT")
        nc.scalar.copy(sT[:], sT_ps[:])
        u_ps = pgn.tile([HID, GPC], f32, space="PSUM", tag="hp")
        nc.tensor.matmul(u_ps[:], W_ri[:], sT[:], start=True,
                         stop=not meta["ro_bias"])
        if meta["ro_bias"]:
            nc.tensor.matmul(u_ps[:], b1row[:], ngrow[:], start=False,
                             stop=True)
        u = gnp.tile([HID, GPC], bf16, tag="u")
        nc.scalar.copy(u[:], u_ps[:])
        v_ps = pgn.tile([HID, GPC], f32, space="PSUM", tag="hp")
        nc.tensor.matmul(v_ps[:], W_ro[:], u[:], start=True,
                         stop=not meta["ro_bias"])
        if meta["ro_bias"]:
            nc.tensor.matmul(v_ps[:], b2row[:], ngrow[:], start=False,
                             stop=True)
        hg = gnp.tile([HID, GPC], bf16, tag="hgv")
        nc.scalar.activation(hg[:], v_ps[:], AF.Relu)
        # compound FC
        c1ps = pgn.tile([HID, GPC], f32, space="PSUM", tag="hp")
        nc.tensor.matmul(c1ps[:], Wc1[:], hg[:], start=True, stop=True)
        cv1 = gnp.tile([HID, GPC], bf16, tag="cv1")
        nc.scalar.activation(cv1[:], c1ps[:], AF.Relu, bias=bc1[:])
        c2ps = pgn.tile([HID, GPC], f32, space="PSUM", tag="hp")
        nc.tensor.matmul(c2ps[:], Wc2[:], cv1[:], start=True, stop=True)
        cv2 = gnp.tile([HID, GPC], bf16, tag="cv2")
        nc.scalar.activation(cv2[:], c2ps[:], AF.Relu, bias=bc2[:])
        # CPI head
        zin = [cv2, pmax]
        z2 = []
        for mc in range(2):
            zps = pgn.tile([HID, GPC], f32, space="PSUM", tag="hp")
            for kc in range(2):
                nc.tensor.matmul(zps[:], Wf1[:, kc, mc * HID:(mc + 1) * HID],
                                 zin[kc][:, :GPC], start=(kc == 0),
                                 stop=(kc == 1))
            zt = gnp.tile([HID, GPC], bf16, tag="z2_%d" % mc)
            nc.scalar.activation(zt[:], zps[:], AF.Relu, bias=bf1[:, mc, :])
            z2.append(zt)
        ops = pgn.tile([1, GPC], f32, space="PSUM", tag="hp")
        for kc in range(2):
            nc.tensor.matmul(ops[:], Wf2[:, kc, :], z2[kc][:],
                             start=(kc == 0), stop=(kc == 1))
        ot = wp.tile([1, GPC], f32, tag="ot")
        nc.scalar.activation(ot[:], ops[:], AF.Sigmoid, bias=bf2[:1, :])
        nc.sync.dma_start(out=out_d[:], in_=ot[:])

    nc.compile()
    return nc


def kernel(**inputs):
    shared, percore, meta = _host_prep(inputs)
    nc = _build(shared, meta)
    in_maps = []
    for c in range(NCORES):
        m = dict(shared)
        m.update(percore[c])
        in_maps.append(m)
    res = run_bass_kernel_spmd(nc, in_maps, list(range(NCORES)))
    out = np.concatenate([res.results[c]["out"].reshape(GPC)
                          for c in range(NCORES)])
    return out.reshape(B, 1).astype(np.float32)


if __name__ == "__main__":
    sys.path.insert(0, "/root/problem")
    import jax
    import reference
    with jax.default_device(jax.devices("cpu")[0]):
        inputs = {k: np.asarray(v) for k, v in reference.setup_inputs().items()}
        exp = np.asarray(reference.reference(**inputs))
    got = kernel(**inputs)
    err = np.abs(got - exp).max()
    rel = err / max(np.abs(exp).max(), 1e-9)
    print("max abs err:", err, " rel:", rel)


# revision 33
# speedup vs baseline: 3.1600x; 1.0174x over previous
"""CPI_DGLLife kernel for 8 Trainium2 NeuronCores (SPMD).

GCN over a 65536-node graph + protein conv1d branch + CPI head.
Sharding: data-parallel over the 512-graph batch (64 graphs / core).

v2 design (vs baseline):
- Single gather stream: bf16 node table packed as 32768 x 512B granules
  (2 node rows each) so int16 indices cover all 65536 nodes. Exact
  per-edge tokens (no 3-table split padding); pad slots carry weight 0.
- Readout commuted past the per-graph segment sum: W_ro_in/W_ro_out are
  applied to the [64, 128] graph sums instead of per node (linear ops
  commute with segment_sum; node-count bias term handled separately).
- bf16 everywhere on the matmul path (fp32 PSUM accumulate), which
  halves gather HBM traffic and runs matmuls at 1 cycle/row.
- Conv restructured: layer-1 folds embed@K1 into a 75-row tap-stacked
  one-hot matmul (1 pass instead of 3); acts split Scalar/Vector.
"""
import sys
sys.path.insert(0, "/opt/trn_rl_repo")
import contextlib
import numpy as np
import ml_dtypes

import concourse.bass as bass
import concourse.bacc as bacc
import concourse.tile as tile
from concourse import mybir
from concourse.bass_utils import run_bass_kernel_spmd
from concourse.masks import make_identity

dt = mybir.dt
AF = mybir.ActivationFunctionType
ALU = mybir.AluOpType
AX = mybir.AxisListType
BF16 = np.dtype(ml_dtypes.bfloat16)

P = 128
N, E, B, L = 65536, 262144, 512, 1000
IN_DIM, HID, VOCAB = 74, 128, 25
CHANNELS = [HID, 96, 128, IN_DIM, HID]
NCORES = 8
GPC = B // NCORES              # graphs per core = 64
PPC = GPC                      # proteins per core = 64
CB = 24                        # gather chunk budget (128-token blocks)
LCONV = 1002                   # 1000 + 2 guard cols


# ------------------------------------------------------------------ host prep
def _host_prep(inputs):
    graph_ids = np.asarray(inputs["graph_ids"])
    src = np.concatenate([np.asarray(inputs["edge_src"]).astype(np.int64),
                          np.arange(N, dtype=np.int64)])
    dst = np.concatenate([np.asarray(inputs["edge_dst"]).astype(np.int64),
                          np.arange(N, dtype=np.int64)])
    deg_out = np.bincount(src, minlength=N).astype(np.float32)
    deg_in = np.bincount(dst, minlength=N).astype(np.float32)

    core_node_lo = np.searchsorted(graph_ids, np.arange(0, B + 1, GPC))
    ncore_nodes = core_node_lo[1:] - core_node_lo[:-1]
    NT = int(np.ceil(ncore_nodes.max() / P))
    NPAD = NT * P

    # per-core degree-sorted node permutation (padded with -1)
    perm = np.full((NCORES, NPAD), -1, np.int64)
    for c in range(NCORES):
        lo, hi = int(core_node_lo[c]), int(core_node_lo[c + 1])
        order = np.argsort(-deg_in[lo:hi], kind="stable") + lo
        perm[c, :hi - lo] = order

    # S tiles: [NT, P, GPC] graph membership of permuted nodes
    S = np.zeros((NCORES, NT, P, GPC), np.float32)
    for c in range(NCORES):
        pm = perm[c]
        valid = pm >= 0
        g = graph_ids[pm[valid]] - c * GPC
        tt = np.arange(NPAD)[valid] // P
        pp = np.arange(NPAD)[valid] % P
        S[c, tt, pp, g] = 1.0

    # node -> (core, tile-position) in permuted order
    pos_of = np.full(N, -1, np.int64)
    core_of = np.full(N, -1, np.int64)
    for c in range(NCORES):
        pm = perm[c]
        v = pm >= 0
        pos_of[pm[v]] = np.arange(NPAD)[v]
        core_of[pm[v]] = c

    # edge -> (core, tile, lane) from dst; slot = rank within (c,t,p)
    ec = core_of[dst]
    et = pos_of[dst] // P
    ep = pos_of[dst] % P
    key = (ec * NT + et) * P + ep
    order = np.argsort(key, kind="stable")
    ks = key[order]
    starts = np.r_[0, np.flatnonzero(np.diff(ks)) + 1]
    grp_len = np.diff(np.r_[starts, len(ks)])
    slot_sorted = np.arange(len(ks)) - np.repeat(starts, grp_len)
    slot = np.empty(len(ks), np.int64)
    slot[order] = slot_sorted
    cnt = np.zeros(NCORES * NT * P, np.int64)
    uk, uc = np.unique(ks, return_counts=True)
    cnt[uk] = uc
    kmax = cnt.reshape(NCORES, NT, P).max(axis=(0, 2))  # [NT] shared schedule
    Bpref = np.r_[0, np.cumsum(kmax)]
    NBLK = int(Bpref[-1])
    NTOK = NBLK * P

    # tile order in the token stream: ascending kmax (small tiles first so
    # the first gather chunks are small and land early)
    tile_order = [t for t in range(NT)][::-1]
    Bof = np.zeros(NT, np.int64)
    off = 0
    for t in tile_order:
        Bof[t] = off
        off += kmax[t]
    assert off == NBLK

    # token (t, slot, lane) -> flat position; idx = src granule, pad -> 0
    tokpos = (Bof[et] + slot) * P + ep
    idx_flat = np.zeros((NCORES, NTOK), np.int16)
    idx_flat[ec, tokpos] = (src >> 1).astype(np.int16)
    # per-token 2-half weights: deg product at the src half, 1e30 (w~0) else
    dval = np.full((NCORES, P, NBLK, 2), 1e30, np.float32)
    dval[ec, ep, Bof[et] + slot, src & 1] = deg_out[src] * deg_in[dst]

    def wrap(a):  # token-major [NCORES, NTOK] -> wrapped [NCORES, 128, NTOK//16]
        ncol = a.shape[1] // 16
        w = a.reshape(NCORES, ncol, 16).transpose(0, 2, 1)
        return np.ascontiguousarray(np.tile(w, (1, 8, 1)))

    idx_wrapped = wrap(idx_flat)

    # gather chunks: pack whole tiles per instruction, ramped budgets so the
    # first chunks finish their descriptor-gen + transfer quickly
    budgets = [8, 8, 8, 8, 16, 16, 16, 16]
    chunks = []
    cur, cb, b0 = [], 0, 0
    for t in tile_order:
        k = int(kmax[t])
        if k == 0:
            continue
        budget = budgets[len(chunks)] if len(chunks) < len(budgets) else CB
        if cb + k > budget and cur:
            chunks.append((cur, b0, cb))
            b0 += cb
            cur, cb = [], 0
        cur.append((t, k))
        cb += k
    if cur:
        chunks.append((cur, b0, cb))

    # bf16 node table: 2 rows (2x 128 cols) per 512B granule
    tab = np.zeros((N, P), BF16)
    tab[:, :IN_DIM] = np.asarray(inputs["node_feats"], np.float32)
    tabg = np.ascontiguousarray(tab.reshape(N // 2, 2 * P))

    # tap-stacked protein one-hot: oh3[c, p, 25t+v, j] = [seq[j+t-1] == v]
    seq = np.asarray(inputs["protein_seq"]).reshape(NCORES, PPC, L)
    oh3 = np.zeros((NCORES, PPC, 3 * VOCAB, L), np.float32)
    ci = np.arange(NCORES)[:, None, None]
    pi = np.arange(PPC)[None, :, None]
    for t in range(3):
        j = np.arange(max(0, 1 - t), min(L, L + 1 - t))
        vals = seq[:, :, j + t - 1]
        oh3[ci, pi, VOCAB * t + vals, j[None, None, :]] = 1.0
    oh3 = oh3.reshape(NCORES, PPC // 4, 4, 3 * VOCAB, L)
    oh3 = np.ascontiguousarray(oh3.transpose(0, 1, 3, 2, 4)).reshape(
        NCORES, PPC // 4, 3 * VOCAB, 4 * L).astype(BF16)

    n_g = np.bincount(graph_ids, minlength=B).astype(np.float32)
    n_g = n_g.reshape(NCORES, 1, GPC)

    # pre-arranged for contiguous DMA; graphs padded to 128 cols (FWL)
    S_r = np.zeros((NCORES, P, NT, P), np.float32)
    S_r[:, :, :, :GPC] = S.transpose(0, 2, 1, 3)

    f32 = np.float32
    shared = {
        "tabg": tabg,
        "W_gc": np.asarray(inputs["W_gc"], f32).astype(BF16),      # [74,128] rhs
        "W_ri": np.asarray(inputs["W_ro_in"], f32).astype(BF16),   # [128,128] lhsT
        "W_ro": np.asarray(inputs["W_ro_out"], f32).astype(BF16),
        "Wc1": np.asarray(inputs["Wc1"], f32).astype(BF16),
        "Wc2": np.asarray(inputs["Wc2"], f32).astype(BF16),
        "embedT": np.ascontiguousarray(
            np.asarray(inputs["embed"], f32).T).astype(BF16),      # [128, 25]
        "Wf1_r": np.ascontiguousarray(
            np.asarray(inputs["Wf1"], f32).reshape(2, HID, 2 * HID)
            .transpose(1, 0, 2)).astype(BF16),                     # [HID,2,2H]
        "bf1_r": np.ascontiguousarray(
            np.asarray(inputs["bf1"], f32).reshape(2, HID, 1)
            .transpose(1, 0, 2)),
        "Wf2_r": np.ascontiguousarray(
            np.asarray(inputs["Wf2"], f32).reshape(2, HID, 1)
            .transpose(1, 0, 2)).astype(BF16),
        "bf2": np.asarray(inputs["bf2"], f32).reshape(1, 1),
        "bc1": np.asarray(inputs["bc1"], f32).reshape(HID, 1),
        "bc2": np.asarray(inputs["bc2"], f32).reshape(HID, 1),
        "bgc_row": np.asarray(inputs["b_gc"], f32).reshape(1, HID).astype(BF16),
        "b1row": np.asarray(inputs["b_ro_in"], f32).reshape(1, HID).astype(BF16),
        "b2row": np.asarray(inputs["b_ro_out"], f32).reshape(1, HID).astype(BF16),
    }
    for l in range(4):
        K = np.asarray(inputs["K%d" % (l + 1)], f32)  # [o, i, 3]
        KT_ = np.ascontiguousarray(K.transpose(1, 2, 0))  # [i, 3, o]
        if KT_.shape[2] < P:  # pad stationary cols to 128 -> enables FWL
            KT_ = np.concatenate(
                [KT_, np.zeros((KT_.shape[0], 3, P - KT_.shape[2]), f32)],
                axis=2)
        shared["K%dT" % (l + 1)] = np.ascontiguousarray(KT_).astype(BF16)
        shared["cb%d" % (l + 1)] = np.asarray(
            inputs["cb%d" % (l + 1)], f32).reshape(-1, 1)

    percore = []
    for c in range(NCORES):
        percore.append({
            "S": np.ascontiguousarray(S_r[c]).astype(BF16),
            "oh3": np.ascontiguousarray(oh3[c]),
            "ixs": idx_wrapped[c],
            "dval": np.ascontiguousarray(dval[c]),
            "ngrow": np.ascontiguousarray(n_g[c]),
        })
    gc_bias = bool(np.any(np.asarray(inputs["b_gc"]) != 0))
    ro_bias = bool(np.any(np.asarray(inputs["b_ro_in"]) != 0)
                   or np.any(np.asarray(inputs["b_ro_out"]) != 0))
    meta = dict(NT=NT, NBLK=NBLK, NTOK=NTOK, chunks=chunks,
                gc_bias=gc_bias, ro_bias=ro_bias)
    return shared, percore, meta


# --------------------------------------------------------------- device build
def _build(shared, meta):
    NT = meta["NT"]
    NBLK = meta["NBLK"]
    NTOK = meta["NTOK"]
    chunks = meta["chunks"]
    maxblk = max(cb for (_, _, cb) in chunks)

    nc = bacc.Bacc("TRN2", target_bir_lowering=False, debug=False,
                   num_devices=NCORES, num_swdge_queues=4)
    f32, bf16, i16 = dt.float32, dt.bfloat16, dt.int16

    D = {k: nc.dram_tensor(k, list(v.shape), dt.from_np(v.dtype),
                           kind="ExternalInput")
         for k, v in shared.items()}
    D["S"] = nc.dram_tensor("S", [P, NT, P], bf16, kind="ExternalInput")
    D["oh3"] = nc.dram_tensor("oh3", [PPC // 4, 3 * VOCAB, 4 * L], bf16,
                              kind="ExternalInput")
    D["ixs"] = nc.dram_tensor("ixs", [P, NTOK // 16], i16,
                              kind="ExternalInput")
    D["dval"] = nc.dram_tensor("dval", [P, NBLK, 2], f32,
                               kind="ExternalInput")
    D["ngrow"] = nc.dram_tensor("ngrow", [1, GPC], f32,
                                kind="ExternalInput")
    out_d = nc.dram_tensor("out", [1, GPC], f32, kind="ExternalOutput")

    with tile.TileContext(nc) as tc, contextlib.ExitStack() as ctx:
        wp = ctx.enter_context(tc.tile_pool(name="wp", bufs=1))
        cvp = ctx.enter_context(tc.tile_pool(name="cvp", bufs=1))
        gp = ctx.enter_context(tc.tile_pool(name="gp", bufs=1))
        gnp = ctx.enter_context(tc.tile_pool(name="gnp", bufs=2))
        pcv = ctx.enter_context(tc.tile_pool(name="pcv", bufs=5, space="PSUM"))
        pgn = ctx.enter_context(tc.tile_pool(name="pgn", bufs=1, space="PSUM"))
        hgp = ctx.enter_context(tc.tile_pool(name="hgp", bufs=1, space="PSUM"))

        # ---------------- setup: weights to SBUF
        def ld(name, shape, dtype, src=None):
            t = wp.tile(shape, dtype, tag=name)
            nc.sync.dma_start(out=t[:], in_=D[name][:] if src is None else src)
            return t

        # gather-critical loads first so DGE can start immediately
        ixs = ld("ixs", [P, NTOK // 16], i16)
        dvt = ld("dval", [P, NBLK, 2], f32)
        embT = ld("embedT", [HID, VOCAB], bf16)
        KT = [ld("K%dT" % (l + 1), [CHANNELS[l], 3, P], bf16)
              for l in range(4)]
        cb = [ld("cb%d" % (l + 1), [CHANNELS[l + 1], 1], f32)
              for l in range(4)]

        # conv x tiles (ping-pong), guard cols zeroed once (vector queue head)
        xb = []
        for l in range(3):
            pair = []
            for j in range(2):
                t = cvp.tile([CHANNELS[l + 1], LCONV], bf16,
                             tag="xb%d_%d" % (l, j))
                nc.vector.memset(t[:, 0:1], 0.0)
                nc.vector.memset(t[:, LCONV - 1:LCONV], 0.0)
                pair.append(t)
            xb.append(pair)

        # token weights: w = rsqrt(deg_out*deg_in); pad 1e30 -> ~0
        wz = wp.tile([P, NBLK, 2], bf16, tag="wz")
        nc.vector.reciprocal(dvt[:], dvt[:])
        nc.scalar.activation(wz[:], dvt[:], AF.Sqrt)

        # L1 stacked weights: rows 25t..25t+24 = embed @ K1_t^T, 128 cols (FWL)
        L1w = wp.tile([3 * VOCAB, P], bf16, tag="l1w")
        nc.vector.memset(L1w[:, CHANNELS[1]:], 0.0)
        for t in range(3):
            pm = pgn.tile([VOCAB, P], f32, space="PSUM", tag="hp")
            nc.tensor.matmul(pm[:], embT[:], KT[0][:, t, :], start=True,
                             stop=True)
            m1t = gnp.tile([VOCAB, CHANNELS[1]], bf16, tag="m1t")
            nc.scalar.copy(m1t[:], pm[:, :CHANNELS[1]])
            nc.sync.dma_start(out=L1w[VOCAB * t:VOCAB * (t + 1), :CHANNELS[1]],
                              in_=m1t[:])

        chunkmax = wp.tile([P, 2, PPC], f32, tag="chunkmax")
        acc = {}

        # ---------------- gather machinery
        gtiles = {}

        def emit_gather(j):
            tl, b0, nb = chunks[j]
            g = gp.tile([P, maxblk, 2 * P], bf16, tag="g%d" % (j % 4))
            nc.gpsimd.dma_gather(
                out_ap=g[:, :nb, :], in_ap=D["tabg"][:],
                idxs_ap=ixs[:, b0 * 8:(b0 + nb) * 8],
                num_idxs=nb * P, num_idxs_reg=nb * P, elem_size=2 * P,
                single_packet=False, queue_num=j % 4)
            gtiles[j] = g

        gscr = gp.tile([P, IN_DIM, 2 * CB], bf16, tag="gscr")

        def drain_chunk(j):
            tl, b0, nb = chunks[j]
            g = gtiles[j]
            # weighted tokens written d-major so the reduce is unit-stride
            gv = g[:, :nb, :].rearrange("p k (a d) -> p (k a) d", a=2)
            wv = wz[:, b0:b0 + nb, :].rearrange("p k a -> p (k a)")
            nc.vector.tensor_tensor(
                out=gscr[:, :, :2 * nb],
                in0=gv[:, :, :IN_DIM].rearrange("p m d -> p d m"),
                in1=wv[:, None, :].to_broadcast([P, IN_DIM, 2 * nb]),
                op=ALU.mult)
            off = 0
            for (t, k) in tl:
                a = wp.tile([P, IN_DIM], f32, tag="acc%d" % t)
                acc[t] = a
                nc.vector.tensor_reduce(
                    out=a[:],
                    in_=gscr[:, :, 2 * off:2 * (off + k)],
                    axis=AX.X, op=ALU.add)
                off += k
            if j + 4 < len(chunks):
                emit_gather(j + 4)

        for j in range(min(4, len(chunks))):
            emit_gather(j)
        drain_ptr = [0]

        def drain_due(p):
            while (drain_ptr[0] < len(chunks)
                   and p >= 6 + 3 * drain_ptr[0]):
                drain_chunk(drain_ptr[0])
                drain_ptr[0] += 1

        # ---------------- conv protein loop
        for p in range(PPC):
            grp, sub = p // 4, p % 4
            if sub == 0:
                ohg = cvp.tile([3 * VOCAB, 4 * L], bf16, tag="oh%d" % (grp % 2))
                nc.sync.dma_start(out=ohg[:], in_=D["oh3"][grp])
            base = sub * L
            x1, x2, x3 = xb[0][p % 2], xb[1][p % 2], xb[2][p % 2]
            for ch in range(2):
                c0 = ch * 500
                pp = pcv.tile([P, 500], f32, space="PSUM", tag="cps")
                nc.tensor.matmul(pp[:], L1w[:], ohg[:, base + c0:base + c0 + 500],
                                 start=True, stop=True)
                nc.scalar.activation(x1[:, 1 + c0:501 + c0],
                                     pp[:CHANNELS[1], :], AF.Relu,
                                     bias=cb[0][:])
            for ch in range(2):
                c0 = ch * 500
                pp = pcv.tile([P, 500], f32, space="PSUM", tag="cps")
                for t in range(3):
                    nc.tensor.matmul(pp[:], KT[1][:, t, :],
                                     x1[:, c0 + t:c0 + t + 500],
                                     start=(t == 0), stop=(t == 2))
                if p % 2 == 0:
                    nc.vector.tensor_scalar(out=x2[:, 1 + c0:501 + c0],
                                            in0=pp[:], scalar1=cb[1][:],
                                            scalar2=0.0, op0=ALU.add,
                                            op1=ALU.max)
                else:
                    nc.scalar.activation(x2[:, 1 + c0:501 + c0], pp[:],
                                         AF.Relu, bias=cb[1][:])
            for ch in range(2):
                c0 = ch * 500
                pp = pcv.tile([P, 500], f32, space="PSUM", tag="cps")
                for t in range(3):
                    nc.tensor.matmul(pp[:], KT[2][:, t, :],
                                     x2[:, c0 + t:c0 + t + 500],
                                     start=(t == 0), stop=(t == 2))
                nc.scalar.activation(x3[:, 1 + c0:501 + c0],
                                     pp[:CHANNELS[3], :], AF.Relu,
                                     bias=cb[2][:])
            for ch in range(2):
                c0 = ch * 500
                pp = pcv.tile([P, 500], f32, space="PSUM", tag="cps")
                for t in range(3):
                    nc.tensor.matmul(pp[:], KT[3][:, t, :],
                                     x3[:, c0 + t:c0 + t + 500],
                                     start=(t == 0), stop=(t == 2))
                nc.vector.reduce_max(out=chunkmax[:, ch, p:p + 1],
                                     in_=pp[:, :500], axis=AX.X)
            drain_due(p)

        while drain_ptr[0] < len(chunks):
            drain_chunk(drain_ptr[0])
            drain_ptr[0] += 1

        # late loads: needed only by the GNN/readout/head phase
        W_gc = ld("W_gc", [IN_DIM, HID], bf16)
        W_ri = ld("W_ri", [HID, HID], bf16)
        W_ro = ld("W_ro", [HID, HID], bf16)
        Wc1 = ld("Wc1", [HID, HID], bf16)
        Wc2 = ld("Wc2", [HID, HID], bf16)
        Wf1 = ld("Wf1_r", [HID, 2, 2 * HID], bf16)
        bf1 = ld("bf1_r", [HID, 2, 1], f32)
        Wf2 = ld("Wf2_r", [HID, 2, 1], bf16)
        bf2 = ld("bf2", [1, 1], f32)
        bc1 = ld("bc1", [HID, 1], f32)
        bc2 = ld("bc2", [HID, 1], f32)
        bgc_row = ld("bgc_row", [1, HID], bf16)
        b1row = ld("b1row", [1, HID], bf16)
        b2row = ld("b2row", [1, HID], bf16)
        ngrow_f = ld("ngrow", [1, GPC], f32)
        Sg = ld("S", [P, NT, P], bf16)
        ngrow = wp.tile([1, GPC], bf16, tag="ngrow_b")
        nc.scalar.copy(ngrow[:], ngrow_f[:])
        ones1 = wp.tile([1, P], bf16, tag="ones1")
        nc.vector.memset(ones1[:], 1.0)
        ident = wp.tile([P, P], f32, tag="ident")
        make_identity(nc, ident[:])
        identb = wp.tile([GPC, GPC], bf16, tag="identb")
        nc.scalar.copy(identb[:], ident[:GPC, :GPC])

        # pmax = relu(max over positions + cb4)  [128, PPC] bf16
        mxt = wp.tile([P, PPC], f32, tag="mxt")
        nc.vector.tensor_reduce(out=mxt[:],
                                in_=chunkmax[:].rearrange("p c q -> p q c"),
                                axis=AX.X, op=ALU.max)
        pmax = wp.tile([P, PPC], bf16, tag="pmax")
        nc.scalar.activation(pmax[:], mxt[:], AF.Relu, bias=cb[3][:])

        # ---------------- GNN: h = relu(agg @ W_gc + b), graph-sum via S
        hgps = hgp.tile([P, HID], f32, space="PSUM", tag="hg")
        for t in range(NT):
            tp = pgn.tile([IN_DIM, P], f32, space="PSUM", tag="tp")
            nc.tensor.transpose(tp[:], acc[t][:], ident[:])
            aT = gnp.tile([IN_DIM, P], bf16, tag="aT")
            nc.scalar.copy(aT[:], tp[:])
            hp = pgn.tile([P, HID], f32, space="PSUM", tag="hp")
            nc.tensor.matmul(hp[:], aT[:], W_gc[:], start=True,
                             stop=not meta["gc_bias"])
            if meta["gc_bias"]:
                nc.tensor.matmul(hp[:], ones1[:], bgc_row[:], start=False,
                                 stop=True)
            h = gnp.tile([P, HID], bf16, tag="h")
            nc.scalar.activation(h[:], hp[:], AF.Relu)
            nc.tensor.matmul(hgps[:], Sg[:, t, :], h[:], start=(t == 0),
                             stop=(t == NT - 1), skip_group_check=True)

        # readout: hg = relu((sum_h @ W_ri + n_g b1) @ W_ro + n_g b2)
        hg_s = gnp.tile([GPC, HID], bf16, tag="hg_s")
        nc.scalar.copy(hg_s[:], hgps[:GPC, :])
        sT_ps = pgn.tile([HID, GPC], bf16, space="PSUM", tag="tp")
        nc.tensor.transpose(sT_ps[:], hg_s[:], identb[:])
        sT = gnp.tile([HID, GPC], bf16, tag="sT")
        nc.scalar.copy(sT[:], sT_ps[:])
        u_ps = pgn.tile([HID, GPC], f32, space="PSUM", tag="hp")
        nc.tensor.matmul(u_ps[:], W_ri[:], sT[:], start=True,
                         stop=not meta["ro_bias"])
        if meta["ro_bias"]:
            nc.tensor.matmul(u_ps[:], b1row[:], ngrow[:], start=False,
                             stop=True)
        u = gnp.tile([HID, GPC], bf16, tag="u")
        nc.scalar.copy(u[:], u_ps[:])
        v_ps = pgn.tile([HID, GPC], f32, space="PSUM", tag="hp")
        nc.tensor.matmul(v_ps[:], W_ro[:], u[:], start=True,
                         stop=not meta["ro_bias"])
        if meta["ro_bias"]:
            nc.tensor.matmul(v_ps[:], b2row[:], ngrow[:], start=False,
                             stop=True)
        hg = gnp.tile([HID, GPC], bf16, tag="hgv")
        nc.scalar.activation(hg[:], v_ps[:], AF.Relu)
        # compound FC
        c1ps = pgn.tile([HID, GPC], f32, space="PSUM", tag="hp")
        nc.tensor.matmul(c1ps[:], Wc1[:], hg[:], start=True, stop=True)
        cv1 = gnp.tile([HID, GPC], bf16, tag="cv1")
        nc.scalar.activation(cv1[:], c1ps[:], AF.Relu, bias=bc1[:])
        c2ps = pgn.tile([HID, GPC], f32, space="PSUM", tag="hp")
        nc.tensor.matmul(c2ps[:], Wc2[:], cv1[:], start=True, stop=True)
        cv2 = gnp.tile([HID, GPC], bf16, tag="cv2")
        nc.scalar.activation(cv2[:], c2ps[:], AF.Relu, bias=bc2[:])
        # CPI head
        zin = [cv2, pmax]
        z2 = []
        for mc in range(2):
            zps = pgn.tile([HID, GPC], f32, space="PSUM", tag="hp")
            for kc in range(2):
                nc.tensor.matmul(zps[:], Wf1[:, kc, mc * HID:(mc + 1) * HID],
                                 zin[kc][:, :GPC], start=(kc == 0),
                                 stop=(kc == 1))
            zt = gnp.tile([HID, GPC], bf16, tag="z2_%d" % mc)
            nc.scalar.activation(zt[:], zps[:], AF.Relu, bias=bf1[:, mc, :])
            z2.append(zt)
        ops = pgn.tile([1, GPC], f32, space="PSUM", tag="hp")
        for kc in range(2):
            nc.tensor.matmul(ops[:], Wf2[:, kc, :], z2[kc][:],
                             start=(kc == 0), stop=(kc == 1))
        ot = wp.tile([1, GPC], f32, tag="ot")
        nc.scalar.activation(ot[:], ops[:], AF.Sigmoid, bias=bf2[:1, :])
        nc.sync.dma_start(out=out_d[:], in_=ot[:])

    nc.compile()
    return nc


def kernel(**inputs):
    shared, percore, meta = _host_prep(inputs)
    nc = _build(shared, meta)
    in_maps = []
    for c in range(NCORES):
        m = dict(shared)
        m.update(percore[c])
        in_maps.append(m)
    res = run_bass_kernel_spmd(nc, in_maps, list(range(NCORES)))
    out = np.concatenate([res.results[c]["out"].reshape(GPC)
                          for c in range(NCORES)])
    return out.reshape(B, 1).astype(np.float32)


if __name__ == "__main__":
    sys.path.insert(0, "/root/problem")
    import jax
    import reference
    with jax.default_device(jax.devices("cpu")[0]):
        inputs = {k: np.asarray(v) for k, v in reference.setup_inputs().items()}
        exp = np.asarray(reference.reference(**inputs))
    got = kernel(**inputs)
    err = np.abs(got - exp).max()
    rel = err / max(np.abs(exp).max(), 1e-9)
    print("max abs err:", err, " rel:", rel)
